# revision 1
# baseline (speedup 1.0000x reference)
"""Contextual-attention kernel for Trainium2, 8 NeuronCores, SPMD.

Decomposition (validated against the jax reference in numpy):
  scores[l,p] = rn[l] * sum_kk V[kk,l] * Gbox[kk,p]      (matmul1, kk=9*128)
  E = exp(scores - max_l scores)                          (softmax numerator)
  Mz[p,:] = sum_l E[l,p] * [rn[l]*V_lkk[l,:1152], 1]      (matmul2, Z in last col)
  out = col2im(Mz[:, :1152]/Z) * m/9 + fg*(1-m)           (host)

Sharding: core c handles sample c//2, pixel half c%2 (2048 of 4096 pixels).
No collectives; host scatters inputs / gathers outputs.
"""
import sys
for _p in ('/opt/trn_rl_repo',):
    if _p not in sys.path:
        sys.path.insert(0, _p)

import numpy as np

import concourse.bass as bass
import concourse.mybir as mybir
import concourse.tile as tile
from concourse import bacc
from concourse.bass_isa import ReduceOp
from concourse.bass_utils import run_bass_kernel_spmd

EPS = 1e-7
C, H, W = 128, 64, 64
L = H * W                      # 4096
KK = 9 * C                     # 1152
NC_COUNT = 8
HALF = L // 2                  # 2048 pixels per core
NCHUNK = 4                     # p-chunks of 512 per core
CW = 512                       # chunk width (pixels)
LT = 32                        # l-tiles of 128
PT_PER_CORE = 16               # p-tiles of 128 per core
DT_MM = mybir.dt.float32  # exact; float32r needs producer-side rounding
F32 = mybir.dt.float32

_compiled = None


def _build_program():
    nc = bacc.Bacc("TRN2", target_bir_lowering=False, debug=False)
    vslab_d = nc.dram_tensor("vslab", [C, 3 * 66 * 64], F32, kind="ExternalInput").ap()
    rnt_d = nc.dram_tensor("rnt", [C, LT], F32, kind="ExternalInput").ap()
    gsh_d = nc.dram_tensor("gsh", [9, C, HALF], F32, kind="ExternalInput").ap()
    vlkk2_d = nc.dram_tensor("vlkk2", [LT, C, KK + 1], F32, kind="ExternalInput").ap()
    mout_d = nc.dram_tensor("mout", [PT_PER_CORE, C, KK + 1], F32,
                            kind="ExternalOutput").ap()
    ident_d = nc.dram_tensor("ident", [C, C], F32, kind="ExternalInput").ap()
    ones1_d = nc.dram_tensor("ones1", [1, C], F32, kind="ExternalInput").ap()

    with tile.TileContext(nc) as tc:
        with (
            tc.tile_pool(name="const", bufs=1) as cpool,
            tc.tile_pool(name="gpool", bufs=2) as gpool,
            tc.tile_pool(name="sspool", bufs=1) as sspool,
            tc.tile_pool(name="small", bufs=2) as small,
            tc.tile_pool(name="vbufs", bufs=4) as vpool,
            tc.tile_pool(name="mo", bufs=4) as mopool,
            tc.tile_pool(name="ps1", bufs=2, space="PSUM") as ps1,
            tc.tile_pool(name="psm", bufs=2, space="PSUM") as psm,
            tc.tile_pool(name="ps2", bufs=4, space="PSUM") as ps2,
        ):
            vs = cpool.tile([C, 3 * 66 * 64], F32)
            nc.sync.dma_start(out=vs[:], in_=vslab_d[:])
            rnt = cpool.tile([C, LT], F32)
            nc.sync.dma_start(out=rnt[:], in_=rnt_d[:])
            ident = cpool.tile([C, C], F32)
            nc.sync.dma_start(out=ident[:], in_=ident_d[:])
            ones1 = cpool.tile([1, C], F32)
            nc.sync.dma_start(out=ones1[:], in_=ones1_d[:])

            for ch in range(NCHUNK):
                # ---- load G chunk: [128, 9, 512]
                gt = gpool.tile([C, 9, CW], F32, tag="gt")
                for k in range(9):
                    nc.sync.dma_start(out=gt[:, k, :],
                                      in_=gsh_d[k, :, ch * CW:(ch + 1) * CW])

                # ---- matmul1: ss[l, p] for all 32 l-tiles of this chunk
                ss = sspool.tile([C, LT * CW], F32, tag="ss")
                for lt in range(LT):
                    ps = ps1.tile([C, CW], F32, tag="ps1")
                    for k in range(9):
                        di, dj = k // 3, k % 3
                        base = (dj * 66 + 2 * lt + di) * 64
                        lhsT = vs[:, base:base + 128]
                        nc.tensor.matmul(ps[:], lhsT.bitcast(DT_MM),
                                         gt[:, k, :].bitcast(DT_MM),
                                         start=(k == 0), stop=(k == 8))
                    # drain with per-partition rn scale
                    nc.vector.tensor_scalar(
                        out=ss[:, lt * CW:(lt + 1) * CW], in0=ps[:],
                        scalar1=rnt[:, lt:lt + 1], scalar2=None,
                        op0=mybir.AluOpType.mult)

                # ---- max over l (32 tiles then across partitions)
                mrun = small.tile([C, CW], F32, tag="mrun")
                nc.vector.tensor_copy(out=mrun[:], in_=ss[:, 0:CW])
                for lt in range(1, LT):
                    nc.vector.tensor_tensor(out=mrun[:], in0=mrun[:],
                                            in1=ss[:, lt * CW:(lt + 1) * CW],
                                            op=mybir.AluOpType.max)
                # cross-partition max via PE: per 128-px block, transpose,
                # free-axis max, transpose back, ones-broadcast to all partitions
                mb = small.tile([C, CW], F32, tag="mb", name=f"mb_{ch}")
                for b in range(4):
                    tps = psm.tile([C, C], F32, tag="tp", name=f"tp_{ch}_{b}")
                    nc.tensor.transpose(tps[:], mrun[:, b * C:(b + 1) * C], ident[:])
                    tms = small.tile([C, C], F32, tag="tms", name=f"tms_{ch}_{b}")
                    nc.vector.tensor_copy(out=tms[:], in_=tps[:])
                    mcol = small.tile([C, 1], F32, tag="mcol", name=f"mc_{ch}_{b}")
                    nc.vector.tensor_reduce(mcol[:], tms[:],
                                            axis=mybir.AxisListType.XYZW,
                                            op=mybir.AluOpType.max)
                    tp2 = psm.tile([1, C], F32, tag="tp", name=f"tp2_{ch}_{b}")
                    nc.tensor.transpose(tp2[:], mcol[:], ident[:])
                    mrow = small.tile([1, C], F32, tag="mrow", name=f"mr_{ch}_{b}")
                    nc.vector.tensor_copy(out=mrow[:], in_=tp2[:])
                    bps = psm.tile([C, C], F32, tag="tp", name=f"bp_{ch}_{b}")
                    nc.tensor.matmul(bps[:], ones1[:], mrow[:], start=True, stop=True)
                    nc.vector.tensor_copy(out=mb[:, b * C:(b + 1) * C], in_=bps[:])
                mrun = mb

                # ---- exp(ss - m)
                for lt in range(LT):
                    sl = ss[:, lt * CW:(lt + 1) * CW]
                    nc.vector.tensor_tensor(out=sl, in0=sl, in1=mrun[:],
                                            op=mybir.AluOpType.subtract)
                    nc.scalar.activation(sl, sl, mybir.ActivationFunctionType.Exp)

                # ---- matmul2: Mz[p, kk] = sum_l E[l,p] * vlkk2[l,kk]
                for (c0, c1) in ((0, 512), (512, 1024), (1024, KK + 1)):
                    cw = c1 - c0
                    pss = [ps2.tile([C, 512], F32, tag="ps2", name=f"ps2_{ch}_{c0}_{i}")
                           for i in range(4)]
                    for ls in range(LT):
                        vb = vpool.tile([C, 512], F32, tag="vb")
                        nc.sync.dma_start(out=vb[:, :cw], in_=vlkk2_d[ls, :, c0:c1])
                        for pt in range(4):
                            lhsT = ss[:, ls * CW + pt * 128: ls * CW + (pt + 1) * 128]
                            nc.tensor.matmul(pss[pt][:, :cw], lhsT.bitcast(DT_MM),
                                             vb[:, :cw].bitcast(DT_MM),
                                             start=(ls == 0), stop=(ls == LT - 1))
                    for pt in range(4):
                        mo = mopool.tile([C, 512], F32, tag="mo")
                        nc.vector.tensor_copy(out=mo[:, :cw], in_=pss[pt][:, :cw])
                        nc.sync.dma_start(out=mout_d[ch * 4 + pt, :, c0:c1],
                                          in_=mo[:, :cw])
    nc.compile()
    return nc


def _host_prep(fg, m):
    """Per-sample operand tensors. fg [C,H,W] f32, m [1,H,W] f32."""
    bg = fg * (1.0 - m)
    vslab = (np.pad(bg, ((0, 0), (1, 1), (1, 1))) + EPS).astype(np.float32)

    v_lkk = np.empty((L, KK + 1), np.float32)
    for di in range(3):
        for dj in range(3):
            v_lkk[:, (di * 3 + dj) * C:(di * 3 + dj + 1) * C] = \
                vslab[:, di:di + H, dj:dj + W].reshape(C, L).T
    v_lkk[:, KK] = 1.0

    norm2 = np.sum(v_lkk[:, :KK].astype(np.float64) ** 2, axis=1)
    rn = (1.0 / np.sqrt(norm2)).astype(np.float32)
    rnt = np.ascontiguousarray(rn.reshape(LT, C).T)          # [128, 32]

    v_lkk2 = v_lkk.copy()
    v_lkk2[:, :KK] *= rn[:, None]
    vlkk2 = np.ascontiguousarray(v_lkk2.reshape(LT, C, KK + 1))

    fgpad = np.pad(fg, ((0, 0), (1, 1), (1, 1)))
    G = np.empty((9, C, L), np.float32)
    for di in range(3):
        for dj in range(3):
            Z = np.zeros((C, H + 2, W + 2), np.float32)
            Z[:, 1:H + 1, 1:W + 1] = fgpad[:, di:di + H, dj:dj + W]
            B = sum(Z[:, a:a + H, b:b + W] for a in range(3) for b in range(3))
            G[di * 3 + dj] = B.reshape(C, L)
    return vslab, rnt, vlkk2, G


def _host_post(Mpatch, fg, m):
    """col2im + final combine for one sample. Mpatch [L, 1152]."""
    rec = np.zeros((C, H, W), np.float32)
    Mp = Mpatch.reshape(H, W, 9, C)
    for di in range(3):
        for dj in range(3):
            oy, ox = 1 - di, 1 - dj
            ys, ye = max(0, -oy), min(H, H - oy)
            xs, xe = max(0, -ox), min(W, W - ox)
            rec[:, ys:ye, xs:xe] += np.transpose(
                Mp[ys + oy:ye + oy, xs + ox:xe + ox, di * 3 + dj, :], (2, 0, 1))
    return rec * m / 9.0 + fg * (1.0 - m)


def kernel(foreground, mask, _results_hook=None):
    global _compiled
    foreground = np.asarray(foreground, np.float32)
    mask = np.asarray(mask, np.float32)
    B = foreground.shape[0]

    if _compiled is None:
        _compiled = _build_program()
    nc = _compiled

    in_maps = []
    preps = []
    for s in range(B):
        vslab, rnt, vlkk2, G = _host_prep(foreground[s], mask[s])
        # [C,66,66] -> [C, 3(dj), 66, 64]: vs2[c,dj,y,x] = vslab[c,y,x+dj]
        vslab = np.ascontiguousarray(
            np.stack([vslab[:, :, dj:dj + 64] for dj in range(3)], axis=1)
        ).reshape(C, 3 * 66 * 64)
        preps.append((vslab, rnt, vlkk2, G))
    for core in range(NC_COUNT):
        s, h = core // 2, core % 2
        vslab, rnt, vlkk2, G = preps[s]
        in_maps.append({
            "vslab": vslab,
            "rnt": rnt,
            "gsh": np.ascontiguousarray(G[:, :, h * HALF:(h + 1) * HALF]),
            "vlkk2": vlkk2,
            "ident": np.eye(C, dtype=np.float32),
            "ones1": np.ones((1, C), np.float32),
        })

    res = run_bass_kernel_spmd(nc, in_maps, list(range(NC_COUNT)))
    if _results_hook is not None:
        _results_hook(res)

    out = np.empty_like(foreground)
    for s in range(B):
        halves = []
        for h in range(2):
            mo = np.asarray(res.results[2 * s + h]["mout"])      # [16,128,1153]
            halves.append(mo.transpose(0, 1, 2).reshape(HALF, KK + 1))
        Mz = np.concatenate(halves, axis=0)                       # [L, 1153]
        Mpatch = Mz[:, :KK] / Mz[:, KK:KK + 1]
        out[s] = _host_post(Mpatch, foreground[s], mask[s])
    return out



# revision 3
# speedup vs baseline: 21.5775x; 21.5775x over previous
"""Contextual-attention kernel for Trainium2 — transfer-minimal version.

The axon tunnel makes host<->device bytes the dominant cost (~235ms call
floor, low effective MB/s). So: upload ONLY fp16 foreground (1MB) +
fp16 mask (8KB) per sample, one sample per core (4 cores), compute all
operand prep on device (background slab, patch norms, 3x3-box G maps,
patch transposes via PE), run scores->softmax->reconstruction, do
col2im on device, download fp16 rec [128,64,64] (1MB). Host does only
the final rec*m/9 + fg*(1-m) combine in f32.

Math (validated against the jax reference):
  scores[l,p] = rn[l] * sum_kk V[kk,l] * Gbox[kk,p]      (matmul1)
  E = exp(scores - max_l scores)
  out9[(di,dj)][c,p] = sum_l (rn[l]*V[l,(di,dj,c)]) * E[l,p]   (matmul2)
  Z[p] = sum_l E[l,p];  rec = col2im(out9 / Z)
"""
import sys
for _p in ('/opt/trn_rl_repo',):
    if _p not in sys.path:
        sys.path.insert(0, _p)

import numpy as np

import concourse.bass as bass
import concourse.mybir as mybir
import concourse.tile as tile
from concourse import bacc
from concourse.bass_utils import run_bass_kernel_spmd

EPS = 1e-7
C, H, W = 128, 64, 64
L = H * W                      # 4096
CW = 512                       # pixel-chunk width (8 image rows)
NCHUNK = L // CW               # 8 chunks (full sample per core)
LT = 32                        # l-tiles of 128
F32 = mybir.dt.float32
F16 = mybir.dt.float16
AX = mybir.AxisListType.XYZW
OP = mybir.AluOpType

_compiled = None


def _build_program():
    nc = bacc.Bacc("TRN2", target_bir_lowering=False, debug=False)
    fg16_d = nc.dram_tensor("fg16", [C, H, W], F16, kind="ExternalInput").ap()
    m16_d = nc.dram_tensor("m16", [1, H, W], F16, kind="ExternalInput").ap()
    ident_d = nc.dram_tensor("ident", [C, C], F32, kind="ExternalInput").ap()
    ones1_d = nc.dram_tensor("ones1", [1, C], F32, kind="ExternalInput").ap()
    onesc_d = nc.dram_tensor("onesc", [C, 1], F32, kind="ExternalInput").ap()
    out_d = nc.dram_tensor("out", [C, H, W], F16, kind="ExternalOutput").ap()

    with tile.TileContext(nc) as tc:
        with (
            tc.tile_pool(name="const", bufs=1) as cpool,
            tc.tile_pool(name="pers", bufs=1) as pers,
            tc.tile_pool(name="dram", bufs=1, space="DRAM") as dpool,
            tc.tile_pool(name="pa", bufs=2, space="PSUM") as pa,
            tc.tile_pool(name="pb", bufs=5, space="PSUM") as pb,
            tc.tile_pool(name="pz", bufs=1, space="PSUM") as pz,
        ):
            ident = cpool.tile([C, C], F32)
            nc.sync.dma_start(out=ident[:], in_=ident_d[:])
            ones1 = cpool.tile([1, C], F32)
            nc.sync.dma_start(out=ones1[:], in_=ones1_d[:])
            onesc = cpool.tile([C, 1], F32)
            nc.sync.dma_start(out=onesc[:], in_=onesc_d[:])

            slab3 = pers.tile([C, 3, H + 2, W], F32)   # pad(bg)+EPS, x-shifted
            rnt = pers.tile([C, LT], F32)              # 1/||patch||
            rec = pers.tile([C, H, W], F32)            # col2im accumulator
            nc.vector.memset(rec[:], 0.0)

            tv_d = dpool.tile([LT, C, 9 * C], F32)     # T: rn-scaled patch rows
            g_d = dpool.tile([NCHUNK, C, 9, 8, W], F32)  # box-summed G chunks

            # ================= prep phases (fg32 scoped) =================
            with tc.tile_pool(name="fgp", bufs=1) as fgp:
                fg16 = fgp.tile([C, H, W], F16)
                nc.sync.dma_start(out=fg16[:], in_=fg16_d[:])
                fg32 = fgp.tile([C, H, W], F32)
                nc.vector.tensor_copy(out=fg32[:], in_=fg16[:])

                # ---- bg = fg*(1-m): broadcast (1-m) over partitions via PE
                with tc.tile_pool(name="ph1", bufs=1) as ph1:
                    m16 = ph1.tile([1, H, W], F16)
                    nc.sync.dma_start(out=m16[:], in_=m16_d[:])
                    om32 = ph1.tile([1, H, W], F32)
                    nc.scalar.activation(om32[:], m16[:],
                                         mybir.ActivationFunctionType.Copy,
                                         bias=1.0, scale=-1.0)
                    omb = ph1.tile([C, H, W], F32)
                    for j in range(8):
                        ps = pa.tile([C, 8, W], F32, tag="pa", name=f"ob_{j}")
                        nc.tensor.matmul(ps[:], ones1[:],
                                         om32[:, 8 * j:8 * (j + 1), :],
                                         start=True, stop=True)
                        nc.vector.tensor_copy(
                            out=omb[:, 8 * j:8 * (j + 1), :], in_=ps[:])
                    bg = ph1.tile([C, H, W], F32)
                    nc.vector.tensor_tensor(out=bg[:], in0=fg32[:], in1=omb[:],
                                            op=OP.mult)

                    # slab3[c,dj,yk,x] = vslab[c,yk,x+dj], vslab=pad(bg)+EPS
                    nc.vector.memset(slab3[:], EPS)
                    nc.vector.tensor_scalar(
                        out=slab3[:, 0, 1:H + 1, 1:64], in0=bg[:, :, 0:63],
                        scalar1=EPS, scalar2=None, op0=OP.add)
                    nc.vector.tensor_scalar(
                        out=slab3[:, 1, 1:H + 1, :], in0=bg[:, :, :],
                        scalar1=EPS, scalar2=None, op0=OP.add)
                    nc.vector.tensor_scalar(
                        out=slab3[:, 2, 1:H + 1, 0:63], in0=bg[:, :, 1:64],
                        scalar1=EPS, scalar2=None, op0=OP.add)

                # ---- patch norms -> rnt
                with tc.tile_pool(name="ph2", bufs=1) as ph2:
                    sq3 = ph2.tile([C, 3, H + 2, W], F32)
                    nc.scalar.square(out=sq3[:], in_=slab3[:])
                    acc = ph2.tile([C, H, W], F32)
                    nc.vector.tensor_copy(out=acc[:], in_=sq3[:, 0, 0:H, :])
                    for dj in range(3):
                        for di in range(3):
                            if dj == 0 and di == 0:
                                continue
                            nc.vector.tensor_tensor(
                                out=acc[:], in0=acc[:],
                                in1=sq3[:, dj, di:di + H, :], op=OP.add)
                    nrm = ph2.tile([C, LT], F32)
                    for lt in range(LT):
                        tps = pa.tile([C, C], F32, tag="pa", name=f"nt_{lt}")
                        nc.tensor.transpose(
                            tps[:], acc[:, 2 * lt:2 * lt + 2, :], ident[:])
                        tsb = ph2.tile([C, C], F32, tag="ntb")
                        nc.vector.tensor_copy(out=tsb[:], in_=tps[:])
                        nc.vector.tensor_reduce(nrm[:, lt:lt + 1], tsb[:],
                                                axis=AX, op=OP.add)
                    nc.scalar.sqrt(out=nrm[:], in_=nrm[:])
                    nc.vector.reciprocal(out=rnt[:], in_=nrm[:])

                # ---- T tiles: rn * transpose(slab patch rows) -> DRAM
                with tc.tile_pool(name="ph3", bufs=4) as ph3:
                    for lt in range(LT):
                        for k in range(9):
                            di, dj = k // 3, k % 3
                            tps = pa.tile([C, C], F32, tag="pa",
                                          name=f"tt_{lt}_{k}")
                            nc.tensor.transpose(
                                tps[:],
                                slab3[:, dj, 2 * lt + di:2 * lt + di + 2, :],
                                ident[:])
                            tsb = ph3.tile([C, C], F32, tag="tsb")
                            nc.vector.tensor_scalar(
                                out=tsb[:], in0=tps[:],
                                scalar1=rnt[:, lt:lt + 1], scalar2=None,
                                op0=OP.mult)
                            nc.sync.dma_start(
                                out=tv_d[lt, :, k * C:(k + 1) * C], in_=tsb[:])

                # ---- G: H-pass canvases, per-chunk V-pass -> DRAM
                with tc.tile_pool(name="ph4", bufs=1) as ph4:
                    hcv = ph4.tile([C, 3, H + 2, W], F32)
                    nc.vector.memset(hcv[:], 0.0)
                    sdj = ph4.tile([C, H, W], F32, tag="sdj")
                    for dj in range(3):
                        if dj == 1:
                            src = fg32
                        else:
                            src = sdj
                            if dj == 0:
                                nc.vector.memset(sdj[:, :, 0:1], 0.0)
                                nc.vector.tensor_copy(out=sdj[:, :, 1:64],
                                                      in_=fg32[:, :, 0:63])
                            else:
                                nc.vector.memset(sdj[:, :, 63:64], 0.0)
                                nc.vector.tensor_copy(out=sdj[:, :, 0:63],
                                                      in_=fg32[:, :, 1:64])
                        # H-taps (x-position clipped) into canvas rows 1..64
                        nc.vector.tensor_copy(out=hcv[:, dj, 1:H + 1, :],
                                              in_=src[:])
                        nc.vector.tensor_tensor(
                            out=hcv[:, dj, 1:H + 1, 1:64],
                            in0=hcv[:, dj, 1:H + 1, 1:64],
                            in1=src[:, :, 0:63], op=OP.add)
                        nc.vector.tensor_tensor(
                            out=hcv[:, dj, 1:H + 1, 0:63],
                            in0=hcv[:, dj, 1:H + 1, 0:63],
                            in1=src[:, :, 1:64], op=OP.add)
                    with tc.tile_pool(name="gph", bufs=1) as gph:
                        for ch in range(NCHUNK):
                            r0 = 8 * ch
                            gt = gph.tile([C, 9, 8, W], F32, tag="gt")
                            for k in range(9):
                                di, dj = k // 3, k % 3
                                nc.vector.tensor_copy(
                                    out=gt[:, k, :, :],
                                    in_=hcv[:, dj, r0 + di:r0 + di + 8, :])
                                for u in (-1, 1):
                                    dylo = max(0, -u - r0)
                                    dyhi = min(7, 63 - u - r0)
                                    if dylo > dyhi:
                                        continue
                                    nc.vector.tensor_tensor(
                                        out=gt[:, k, dylo:dyhi + 1, :],
                                        in0=gt[:, k, dylo:dyhi + 1, :],
                                        in1=hcv[:, dj,
                                                r0 + dylo + u + di:
                                                r0 + dyhi + u + di + 1, :],
                                        op=OP.add)
                            nc.sync.dma_start(out=g_d[ch, :, :, :, :],
                                              in_=gt[:])

            # ================= main loop over pixel chunks =================
            with (
                tc.tile_pool(name="sspool", bufs=1) as sspool,
                tc.tile_pool(name="gmain", bufs=1) as gmain,
                tc.tile_pool(name="vpool", bufs=3) as vpool,
                tc.tile_pool(name="small", bufs=2) as small,
                tc.tile_pool(name="mo", bufs=4) as mopool,
            ):
                for ch in range(NCHUNK):
                    r0 = 8 * ch
                    gt = gmain.tile([C, 9, 8, W], F32, tag="gt")
                    nc.sync.dma_start(out=gt[:], in_=g_d[ch, :, :, :, :])

                    # matmul1: scores for all 32 l-tiles at this chunk
                    ss = sspool.tile([C, LT, CW], F32, tag="ss")
                    for lt in range(LT):
                        ps = pa.tile([C, CW], F32, tag="pa",
                                     name=f"m1_{ch}_{lt}")
                        for k in range(9):
                            di, dj = k // 3, k % 3
                            lhsT = slab3[:, dj, 2 * lt + di:2 * lt + di + 2, :]
                            nc.tensor.matmul(ps[:], lhsT, gt[:, k, :, :],
                                             start=(k == 0), stop=(k == 8))
                        nc.vector.tensor_scalar(
                            out=ss[:, lt, :], in0=ps[:],
                            scalar1=rnt[:, lt:lt + 1], scalar2=None,
                            op0=OP.mult)

                    # max over l: 32 tiles then across partitions via PE
                    mrun = small.tile([C, CW], F32, tag="mrun")
                    nc.vector.tensor_copy(out=mrun[:], in_=ss[:, 0, :])
                    for lt in range(1, LT):
                        nc.vector.tensor_tensor(out=mrun[:], in0=mrun[:],
                                                in1=ss[:, lt, :], op=OP.max)
                    mb = small.tile([C, CW], F32, tag="mb", name=f"mb_{ch}")
                    for b in range(4):
                        tps = pa.tile([C, C], F32, tag="pa",
                                      name=f"tp_{ch}_{b}")
                        nc.tensor.transpose(tps[:], mrun[:, b * C:(b + 1) * C],
                                            ident[:])
                        tms = small.tile([C, C], F32, tag="tms",
                                         name=f"tms_{ch}_{b}")
                        nc.vector.tensor_copy(out=tms[:], in_=tps[:])
                        mcol = small.tile([C, 1], F32, tag="mcol",
                                          name=f"mc_{ch}_{b}")
                        nc.vector.tensor_reduce(mcol[:], tms[:], axis=AX,
                                                op=OP.max)
                        tp2 = pa.tile([1, C], F32, tag="pa",
                                      name=f"t2_{ch}_{b}")
                        nc.tensor.transpose(tp2[:], mcol[:], ident[:])
                        mrow = small.tile([1, C], F32, tag="mrow",
                                          name=f"mr_{ch}_{b}")
                        nc.vector.tensor_copy(out=mrow[:], in_=tp2[:])
                        bps = pa.tile([C, C], F32, tag="pa",
                                      name=f"bp_{ch}_{b}")
                        nc.tensor.matmul(bps[:], ones1[:], mrow[:],
                                         start=True, stop=True)
                        nc.vector.tensor_copy(out=mb[:, b * C:(b + 1) * C],
                                              in_=bps[:])

                    # E = exp(ss - mb)
                    for lt in range(LT):
                        sl = ss[:, lt, :]
                        nc.vector.tensor_tensor(out=sl, in0=sl, in1=mb[:],
                                                op=OP.subtract)
                        nc.scalar.activation(sl, sl,
                                             mybir.ActivationFunctionType.Exp)

                    # matmul2: out9 planes [c,p] + Z, two PSUM passes
                    mos = []
                    zrow = pz.tile([1, CW], F32, tag="z", name=f"z_{ch}")
                    rzb = None
                    for pass_i, planes in enumerate(((0, 1, 2, 3, 4),
                                                     (5, 6, 7, 8))):
                        c0 = planes[0] * C
                        c1 = (planes[-1] + 1) * C
                        pts = [pb.tile([C, 8, W], F32, tag="o",
                                       name=f"o_{ch}_{j}") for j in planes]
                        for ls in range(LT):
                            vb = vpool.tile([C, 5 * C], F32, tag="vb")
                            nc.sync.dma_start(out=vb[:, :c1 - c0],
                                              in_=tv_d[ls, :, c0:c1])
                            for i, j in enumerate(planes):
                                nc.tensor.matmul(
                                    pts[i][:], vb[:, i * C:(i + 1) * C],
                                    ss[:, ls, :],
                                    start=(ls == 0), stop=(ls == LT - 1))
                            if pass_i == 0:
                                nc.tensor.matmul(zrow[:], onesc[:],
                                                 ss[:, ls, :],
                                                 start=(ls == 0),
                                                 stop=(ls == LT - 1))
                        if pass_i == 0:
                            zs = small.tile([1, CW], F32, tag="zs",
                                            name=f"zs_{ch}")
                            nc.vector.tensor_copy(out=zs[:], in_=zrow[:])
                            rz = small.tile([1, CW], F32, tag="rz",
                                            name=f"rz_{ch}")
                            nc.vector.reciprocal(out=rz[:], in_=zs[:])
                            bps = pa.tile([C, 8, W], F32, tag="pa",
                                          name=f"zb_{ch}")
                            nc.tensor.matmul(bps[:], ones1[:], rz[:],
                                             start=True, stop=True)
                            rzb = small.tile([C, 8, W], F32, tag="rzb",
                                             name=f"rzb_{ch}")
                            nc.vector.tensor_copy(out=rzb[:], in_=bps[:])
                        for i, j in enumerate(planes):
                            mo = mopool.tile([C, 8, W], F32, tag="mo",
                                             name=f"mo_{ch}_{j}")
                            nc.vector.tensor_tensor(out=mo[:], in0=pts[i][:],
                                                    in1=rzb[:], op=OP.mult)
                            mos.append((j, mo))

                    # col2im: rec[y,x] += mo[(di,dj)][y+1-di, x+1-dj]
                    for j, mo in mos:
                        di, dj = j // 3, j % 3
                        ylo = max(0, r0 + di - 1)
                        yhi = min(63, r0 + di + 6)
                        if ylo > yhi:
                            continue
                        sy = ylo - (r0 + di - 1)
                        nr = yhi - ylo + 1
                        if dj == 0:
                            xd, xs, ncol = 0, 1, 63
                        elif dj == 1:
                            xd, xs, ncol = 0, 0, 64
                        else:
                            xd, xs, ncol = 1, 0, 63
                        nc.vector.tensor_tensor(
                            out=rec[:, ylo:ylo + nr, xd:xd + ncol],
                            in0=rec[:, ylo:ylo + nr, xd:xd + ncol],
                            in1=mo[:, sy:sy + nr, xs:xs + ncol],
                            op=OP.add)

                # ---- fp16 cast + output
                out16 = small.tile([C, H, W], F16, tag="o16")
                nc.vector.tensor_copy(out=out16[:], in_=rec[:])
                nc.sync.dma_start(out=out_d[:], in_=out16[:])
    nc.compile()
    return nc


def make_in_maps(foreground, mask):
    foreground = np.asarray(foreground, np.float32)
    mask = np.asarray(mask, np.float32)
    ident = np.eye(C, dtype=np.float32)
    ones1 = np.ones((1, C), np.float32)
    onesc = np.ones((C, 1), np.float32)
    in_maps = []
    for s in range(foreground.shape[0]):
        in_maps.append({
            "fg16": foreground[s].astype(np.float16),
            "m16": mask[s].astype(np.float16),
            "ident": ident,
            "ones1": ones1,
            "onesc": onesc,
        })
    return in_maps


def kernel(foreground, mask, _results_hook=None):
    global _compiled
    foreground = np.asarray(foreground, np.float32)
    mask = np.asarray(mask, np.float32)
    B = foreground.shape[0]

    if _compiled is None:
        _compiled = _build_program()
    nc = _compiled

    in_maps = make_in_maps(foreground, mask)
    res = run_bass_kernel_spmd(nc, in_maps, list(range(B)))
    if _results_hook is not None:
        _results_hook(res)

    out = np.empty_like(foreground)
    for s in range(B):
        rec = np.asarray(res.results[s]["out"]).astype(np.float32)
        m = mask[s]
        out[s] = rec * m / 9.0 + foreground[s] * (1.0 - m)
    return out


# revision 6
# speedup vs baseline: 21.8655x; 1.0133x over previous
"""Contextual-attention kernel for Trainium2 — transfer-minimal version.

The axon tunnel makes host<->device bytes the dominant cost (~235ms call
floor, low effective MB/s). So: upload ONLY fp16 foreground (1MB) +
fp16 mask (8KB) per sample, one sample per core (4 cores), compute all
operand prep on device (background slab, patch norms, 3x3-box G maps,
patch transposes via PE), run scores->softmax->reconstruction, do
col2im on device, download fp16 rec [128,64,64] (1MB). Host does only
the final rec*m/9 + fg*(1-m) combine in f32.

Math (validated against the jax reference):
  scores[l,p] = rn[l] * sum_kk V[kk,l] * Gbox[kk,p]      (matmul1)
  E = exp(scores - max_l scores)
  out9[(di,dj)][c,p] = sum_l (rn[l]*V[l,(di,dj,c)]) * E[l,p]   (matmul2)
  Z[p] = sum_l E[l,p];  rec = col2im(out9 / Z)
"""
import sys
for _p in ('/opt/trn_rl_repo',):
    if _p not in sys.path:
        sys.path.insert(0, _p)

import numpy as np

import concourse.bass as bass
import concourse.mybir as mybir
import concourse.tile as tile
from concourse import bacc
from concourse.bass_utils import run_bass_kernel_spmd

EPS = 1e-7
C, H, W = 128, 64, 64
L = H * W                      # 4096
CW = 512                       # pixel-chunk width (8 image rows)
NCHUNK = L // CW               # 8 chunks (full sample per core)
LT = 32                        # l-tiles of 128
F32 = mybir.dt.float32
F16 = mybir.dt.float16
AX = mybir.AxisListType.XYZW
OP = mybir.AluOpType

_compiled = None
_runner = None


def _build_program():
    nc = bacc.Bacc("TRN2", target_bir_lowering=False, debug=False)
    fg16_d = nc.dram_tensor("fg16", [C, H, W], F16, kind="ExternalInput").ap()
    m16_d = nc.dram_tensor("m16", [1, H, W], F16, kind="ExternalInput").ap()
    ident_d = nc.dram_tensor("ident", [C, C], F32, kind="ExternalInput").ap()
    ones1_d = nc.dram_tensor("ones1", [1, C], F32, kind="ExternalInput").ap()
    onesc_d = nc.dram_tensor("onesc", [C, 1], F32, kind="ExternalInput").ap()
    out_d = nc.dram_tensor("out", [C, H, W], F16, kind="ExternalOutput").ap()

    with tile.TileContext(nc) as tc:
        with (
            tc.tile_pool(name="const", bufs=1) as cpool,
            tc.tile_pool(name="pers", bufs=1) as pers,
            tc.tile_pool(name="dram", bufs=1, space="DRAM") as dpool,
            tc.tile_pool(name="pa", bufs=2, space="PSUM") as pa,
            tc.tile_pool(name="pb", bufs=5, space="PSUM") as pb,
            tc.tile_pool(name="pz", bufs=1, space="PSUM") as pz,
        ):
            ident = cpool.tile([C, C], F32)
            nc.sync.dma_start(out=ident[:], in_=ident_d[:])
            ones1 = cpool.tile([1, C], F32)
            nc.sync.dma_start(out=ones1[:], in_=ones1_d[:])
            onesc = cpool.tile([C, 1], F32)
            nc.sync.dma_start(out=onesc[:], in_=onesc_d[:])

            slab3 = pers.tile([C, 3, H + 2, W], F32)   # pad(bg)+EPS, x-shifted
            rnt = pers.tile([C, LT], F32)              # 1/||patch||
            rec = pers.tile([C, H, W], F32)            # col2im accumulator
            nc.vector.memset(rec[:], 0.0)

            tv_d = dpool.tile([LT, C, 9 * C], F32)     # T: rn-scaled patch rows
            g_d = dpool.tile([NCHUNK, C, 9, 8, W], F32)  # box-summed G chunks

            # ================= prep phases (fg32 scoped) =================
            with tc.tile_pool(name="fgp", bufs=1) as fgp:
                fg16 = fgp.tile([C, H, W], F16)
                nc.sync.dma_start(out=fg16[:], in_=fg16_d[:])
                fg32 = fgp.tile([C, H, W], F32)
                nc.vector.tensor_copy(out=fg32[:], in_=fg16[:])

                # ---- bg = fg*(1-m): broadcast (1-m) over partitions via PE
                with tc.tile_pool(name="ph1", bufs=1) as ph1:
                    m16 = ph1.tile([1, H, W], F16)
                    nc.sync.dma_start(out=m16[:], in_=m16_d[:])
                    om32 = ph1.tile([1, H, W], F32)
                    nc.scalar.activation(om32[:], m16[:],
                                         mybir.ActivationFunctionType.Copy,
                                         bias=1.0, scale=-1.0)
                    omb = ph1.tile([C, H, W], F32)
                    for j in range(8):
                        ps = pa.tile([C, 8, W], F32, tag="pa", name=f"ob_{j}")
                        nc.tensor.matmul(ps[:], ones1[:],
                                         om32[:, 8 * j:8 * (j + 1), :],
                                         start=True, stop=True)
                        nc.vector.tensor_copy(
                            out=omb[:, 8 * j:8 * (j + 1), :], in_=ps[:])
                    bg = ph1.tile([C, H, W], F32)
                    nc.vector.tensor_tensor(out=bg[:], in0=fg32[:], in1=omb[:],
                                            op=OP.mult)

                    # slab3[c,dj,yk,x] = vslab[c,yk,x+dj], vslab=pad(bg)+EPS
                    nc.vector.memset(slab3[:], EPS)
                    nc.vector.tensor_scalar(
                        out=slab3[:, 0, 1:H + 1, 1:64], in0=bg[:, :, 0:63],
                        scalar1=EPS, scalar2=None, op0=OP.add)
                    nc.vector.tensor_scalar(
                        out=slab3[:, 1, 1:H + 1, :], in0=bg[:, :, :],
                        scalar1=EPS, scalar2=None, op0=OP.add)
                    nc.vector.tensor_scalar(
                        out=slab3[:, 2, 1:H + 1, 0:63], in0=bg[:, :, 1:64],
                        scalar1=EPS, scalar2=None, op0=OP.add)

                # ---- patch norms -> rnt
                with tc.tile_pool(name="ph2", bufs=1) as ph2:
                    sq3 = ph2.tile([C, 3, H + 2, W], F32)
                    nc.scalar.square(out=sq3[:], in_=slab3[:])
                    acc = ph2.tile([C, H, W], F32)
                    nc.vector.tensor_copy(out=acc[:], in_=sq3[:, 0, 0:H, :])
                    for dj in range(3):
                        for di in range(3):
                            if dj == 0 and di == 0:
                                continue
                            nc.vector.tensor_tensor(
                                out=acc[:], in0=acc[:],
                                in1=sq3[:, dj, di:di + H, :], op=OP.add)
                    nrm = ph2.tile([C, LT], F32)
                    for lt in range(LT):
                        tps = pa.tile([C, C], F32, tag="pa", name=f"nt_{lt}")
                        nc.tensor.transpose(
                            tps[:], acc[:, 2 * lt:2 * lt + 2, :], ident[:])
                        tsb = ph2.tile([C, C], F32, tag="ntb")
                        nc.vector.tensor_copy(out=tsb[:], in_=tps[:])
                        nc.vector.tensor_reduce(nrm[:, lt:lt + 1], tsb[:],
                                                axis=AX, op=OP.add)
                    nc.scalar.sqrt(out=nrm[:], in_=nrm[:])
                    nc.vector.reciprocal(out=rnt[:], in_=nrm[:])

                # ---- T tiles: rn * transpose(slab patch rows) -> DRAM
                with tc.tile_pool(name="ph3", bufs=4) as ph3:
                    for lt in range(LT):
                        for k in range(9):
                            di, dj = k // 3, k % 3
                            tps = pa.tile([C, C], F32, tag="pa",
                                          name=f"tt_{lt}_{k}")
                            nc.tensor.transpose(
                                tps[:],
                                slab3[:, dj, 2 * lt + di:2 * lt + di + 2, :],
                                ident[:])
                            tsb = ph3.tile([C, C], F32, tag="tsb")
                            nc.vector.tensor_scalar(
                                out=tsb[:], in0=tps[:],
                                scalar1=rnt[:, lt:lt + 1], scalar2=None,
                                op0=OP.mult)
                            nc.sync.dma_start(
                                out=tv_d[lt, :, k * C:(k + 1) * C], in_=tsb[:])

                # ---- G: H-pass canvases, per-chunk V-pass -> DRAM
                with tc.tile_pool(name="ph4", bufs=1) as ph4:
                    hcv = ph4.tile([C, 3, H + 2, W], F32)
                    nc.vector.memset(hcv[:], 0.0)
                    sdj = ph4.tile([C, H, W], F32, tag="sdj")
                    for dj in range(3):
                        if dj == 1:
                            src = fg32
                        else:
                            src = sdj
                            if dj == 0:
                                nc.vector.memset(sdj[:, :, 0:1], 0.0)
                                nc.vector.tensor_copy(out=sdj[:, :, 1:64],
                                                      in_=fg32[:, :, 0:63])
                            else:
                                nc.vector.memset(sdj[:, :, 63:64], 0.0)
                                nc.vector.tensor_copy(out=sdj[:, :, 0:63],
                                                      in_=fg32[:, :, 1:64])
                        # H-taps (x-position clipped) into canvas rows 1..64
                        nc.vector.tensor_copy(out=hcv[:, dj, 1:H + 1, :],
                                              in_=src[:])
                        nc.vector.tensor_tensor(
                            out=hcv[:, dj, 1:H + 1, 1:64],
                            in0=hcv[:, dj, 1:H + 1, 1:64],
                            in1=src[:, :, 0:63], op=OP.add)
                        nc.vector.tensor_tensor(
                            out=hcv[:, dj, 1:H + 1, 0:63],
                            in0=hcv[:, dj, 1:H + 1, 0:63],
                            in1=src[:, :, 1:64], op=OP.add)
                    with tc.tile_pool(name="gph", bufs=1) as gph:
                        for ch in range(NCHUNK):
                            r0 = 8 * ch
                            gt = gph.tile([C, 9, 8, W], F32, tag="gt")
                            for k in range(9):
                                di, dj = k // 3, k % 3
                                nc.vector.tensor_copy(
                                    out=gt[:, k, :, :],
                                    in_=hcv[:, dj, r0 + di:r0 + di + 8, :])
                                for u in (-1, 1):
                                    dylo = max(0, -u - r0)
                                    dyhi = min(7, 63 - u - r0)
                                    if dylo > dyhi:
                                        continue
                                    nc.vector.tensor_tensor(
                                        out=gt[:, k, dylo:dyhi + 1, :],
                                        in0=gt[:, k, dylo:dyhi + 1, :],
                                        in1=hcv[:, dj,
                                                r0 + dylo + u + di:
                                                r0 + dyhi + u + di + 1, :],
                                        op=OP.add)
                            nc.sync.dma_start(out=g_d[ch, :, :, :, :],
                                              in_=gt[:])

            # ================= main loop over pixel chunks =================
            with (
                tc.tile_pool(name="sspool", bufs=1) as sspool,
                tc.tile_pool(name="gmain", bufs=1) as gmain,
                tc.tile_pool(name="vpool", bufs=3) as vpool,
                tc.tile_pool(name="small", bufs=2) as small,
                tc.tile_pool(name="mo", bufs=4) as mopool,
            ):
                for ch in range(NCHUNK):
                    r0 = 8 * ch
                    gt = gmain.tile([C, 9, 8, W], F32, tag="gt")
                    nc.sync.dma_start(out=gt[:], in_=g_d[ch, :, :, :, :])

                    # matmul1: scores for all 32 l-tiles at this chunk
                    ss = sspool.tile([C, LT, CW], F32, tag="ss")
                    for lt in range(LT):
                        ps = pa.tile([C, CW], F32, tag="pa",
                                     name=f"m1_{ch}_{lt}")
                        for k in range(9):
                            di, dj = k // 3, k % 3
                            lhsT = slab3[:, dj, 2 * lt + di:2 * lt + di + 2, :]
                            nc.tensor.matmul(ps[:], lhsT, gt[:, k, :, :],
                                             start=(k == 0), stop=(k == 8))
                        nc.vector.tensor_scalar(
                            out=ss[:, lt, :], in0=ps[:],
                            scalar1=rnt[:, lt:lt + 1], scalar2=None,
                            op0=OP.mult)

                    # max over l: 32 tiles then across partitions via PE
                    mrun = small.tile([C, CW], F32, tag="mrun")
                    nc.vector.tensor_copy(out=mrun[:], in_=ss[:, 0, :])
                    for lt in range(1, LT):
                        nc.vector.tensor_tensor(out=mrun[:], in0=mrun[:],
                                                in1=ss[:, lt, :], op=OP.max)
                    mb = small.tile([C, CW], F32, tag="mb", name=f"mb_{ch}")
                    for b in range(4):
                        tps = pa.tile([C, C], F32, tag="pa",
                                      name=f"tp_{ch}_{b}")
                        nc.tensor.transpose(tps[:], mrun[:, b * C:(b + 1) * C],
                                            ident[:])
                        tms = small.tile([C, C], F32, tag="tms",
                                         name=f"tms_{ch}_{b}")
                        nc.vector.tensor_copy(out=tms[:], in_=tps[:])
                        mcol = small.tile([C, 1], F32, tag="mcol",
                                          name=f"mc_{ch}_{b}")
                        nc.vector.tensor_reduce(mcol[:], tms[:], axis=AX,
                                                op=OP.max)
                        tp2 = pa.tile([1, C], F32, tag="pa",
                                      name=f"t2_{ch}_{b}")
                        nc.tensor.transpose(tp2[:], mcol[:], ident[:])
                        mrow = small.tile([1, C], F32, tag="mrow",
                                          name=f"mr_{ch}_{b}")
                        nc.vector.tensor_copy(out=mrow[:], in_=tp2[:])
                        bps = pa.tile([C, C], F32, tag="pa",
                                      name=f"bp_{ch}_{b}")
                        nc.tensor.matmul(bps[:], ones1[:], mrow[:],
                                         start=True, stop=True)
                        nc.vector.tensor_copy(out=mb[:, b * C:(b + 1) * C],
                                              in_=bps[:])

                    # E = exp(ss - mb)
                    for lt in range(LT):
                        sl = ss[:, lt, :]
                        nc.vector.tensor_tensor(out=sl, in0=sl, in1=mb[:],
                                                op=OP.subtract)
                        nc.scalar.activation(sl, sl,
                                             mybir.ActivationFunctionType.Exp)

                    # matmul2: out9 planes [c,p] + Z, two PSUM passes
                    mos = []
                    zrow = pz.tile([1, CW], F32, tag="z", name=f"z_{ch}")
                    rzb = None
                    for pass_i, planes in enumerate(((0, 1, 2, 3, 4),
                                                     (5, 6, 7, 8))):
                        c0 = planes[0] * C
                        c1 = (planes[-1] + 1) * C
                        pts = [pb.tile([C, 8, W], F32, tag="o",
                                       name=f"o_{ch}_{j}") for j in planes]
                        for ls in range(LT):
                            vb = vpool.tile([C, 5 * C], F32, tag="vb")
                            nc.sync.dma_start(out=vb[:, :c1 - c0],
                                              in_=tv_d[ls, :, c0:c1])
                            for i, j in enumerate(planes):
                                nc.tensor.matmul(
                                    pts[i][:], vb[:, i * C:(i + 1) * C],
                                    ss[:, ls, :],
                                    start=(ls == 0), stop=(ls == LT - 1))
                            if pass_i == 0:
                                nc.tensor.matmul(zrow[:], onesc[:],
                                                 ss[:, ls, :],
                                                 start=(ls == 0),
                                                 stop=(ls == LT - 1))
                        if pass_i == 0:
                            zs = small.tile([1, CW], F32, tag="zs",
                                            name=f"zs_{ch}")
                            nc.vector.tensor_copy(out=zs[:], in_=zrow[:])
                            rz = small.tile([1, CW], F32, tag="rz",
                                            name=f"rz_{ch}")
                            nc.vector.reciprocal(out=rz[:], in_=zs[:])
                            bps = pa.tile([C, 8, W], F32, tag="pa",
                                          name=f"zb_{ch}")
                            nc.tensor.matmul(bps[:], ones1[:], rz[:],
                                             start=True, stop=True)
                            rzb = small.tile([C, 8, W], F32, tag="rzb",
                                             name=f"rzb_{ch}")
                            nc.vector.tensor_copy(out=rzb[:], in_=bps[:])
                        for i, j in enumerate(planes):
                            mo = mopool.tile([C, 8, W], F32, tag="mo",
                                             name=f"mo_{ch}_{j}")
                            nc.vector.tensor_tensor(out=mo[:], in0=pts[i][:],
                                                    in1=rzb[:], op=OP.mult)
                            mos.append((j, mo))

                    # col2im: rec[y,x] += mo[(di,dj)][y+1-di, x+1-dj]
                    for j, mo in mos:
                        di, dj = j // 3, j % 3
                        ylo = max(0, r0 + di - 1)
                        yhi = min(63, r0 + di + 6)
                        if ylo > yhi:
                            continue
                        sy = ylo - (r0 + di - 1)
                        nr = yhi - ylo + 1
                        if dj == 0:
                            xd, xs, ncol = 0, 1, 63
                        elif dj == 1:
                            xd, xs, ncol = 0, 0, 64
                        else:
                            xd, xs, ncol = 1, 0, 63
                        nc.vector.tensor_tensor(
                            out=rec[:, ylo:ylo + nr, xd:xd + ncol],
                            in0=rec[:, ylo:ylo + nr, xd:xd + ncol],
                            in1=mo[:, sy:sy + nr, xs:xs + ncol],
                            op=OP.add)

                # ---- fp16 cast + output
                out16 = small.tile([C, H, W], F16, tag="o16")
                nc.vector.tensor_copy(out=out16[:], in_=rec[:])
                nc.sync.dma_start(out=out_d[:], in_=out16[:])
    nc.compile()
    return nc


def _make_runner(nc, n_cores):
    """Cached-jit SPMD executor: builds the shard_map wrapper ONCE.

    run_bass_kernel_spmd re-traces + re-jits per call (~650ms overhead under
    axon); this path also creates the donated zero output buffers on device
    (sharded jnp.zeros) instead of uploading them from host each call.
    """
    import jax
    import jax.numpy as jnp
    from jax.sharding import Mesh, PartitionSpec, NamedSharding
    try:
        from jax import shard_map
    except ImportError:
        from jax.experimental.shard_map import shard_map
    from concourse.bass2jax import (_bass_exec_p, install_neuronx_cc_hook,
                                    partition_id_tensor)

    install_neuronx_cc_hook()
    partition_name = nc.partition_id_tensor.name if nc.partition_id_tensor else None
    in_names, out_names, out_avals = [], [], []
    for alloc in nc.m.functions[0].allocations:
        if not isinstance(alloc, mybir.MemoryLocationSet):
            continue
        name = alloc.memorylocations[0].name
        if alloc.kind == "ExternalInput":
            if name != partition_name:
                in_names.append(name)
        elif alloc.kind == "ExternalOutput":
            out_names.append(name)
            out_avals.append(jax.core.ShapedArray(
                tuple(alloc.tensor_shape), mybir.dt.np(alloc.dtype)))
    n_params = len(in_names)
    n_outs = len(out_avals)
    all_in_names = list(in_names) + list(out_names)
    if partition_name is not None:
        all_in_names.append(partition_name)

    def _body(*args):
        operands = list(args)
        if partition_name is not None:
            operands.append(partition_id_tensor())
        return tuple(_bass_exec_p.bind(
            *operands, out_avals=tuple(out_avals), in_names=tuple(all_in_names),
            out_names=tuple(out_names), lowering_input_output_aliases=(),
            sim_require_finite=True, sim_require_nnan=True, nc=nc))

    devices = jax.devices()[:n_cores]
    mesh = Mesh(np.asarray(devices), ("core",))
    in_specs = (PartitionSpec("core"),) * (n_params + n_outs)
    out_specs = (PartitionSpec("core"),) * n_outs
    donate = tuple(range(n_params, n_params + n_outs))
    sharded = jax.jit(
        shard_map(_body, mesh=mesh, in_specs=in_specs, out_specs=out_specs,
                  check_rep=False),
        donate_argnums=donate, keep_unused=True)

    zero_shardings = [NamedSharding(mesh, PartitionSpec("core"))] * n_outs

    def _zeros():
        return tuple(jnp.zeros((n_cores * a.shape[0], *a.shape[1:]), a.dtype)
                     for a in out_avals)

    make_zeros = jax.jit(_zeros, out_shardings=tuple(zero_shardings))

    def run(in_maps):
        concat_in = [
            np.concatenate([np.asarray(in_maps[c][nm]) for c in range(n_cores)],
                           axis=0) for nm in in_names]
        out_arrs = sharded(*concat_in, *make_zeros())
        out_arrs = [np.asarray(a) for a in out_arrs]
        return [{nm: out_arrs[i].reshape(n_cores, *out_avals[i].shape)[c]
                 for i, nm in enumerate(out_names)} for c in range(n_cores)]

    return run


def make_in_maps(foreground, mask):
    foreground = np.asarray(foreground, np.float32)
    mask = np.asarray(mask, np.float32)
    ident = np.eye(C, dtype=np.float32)
    ones1 = np.ones((1, C), np.float32)
    onesc = np.ones((C, 1), np.float32)
    in_maps = []
    for s in range(foreground.shape[0]):
        in_maps.append({
            "fg16": foreground[s].astype(np.float16),
            "m16": mask[s].astype(np.float16),
            "ident": ident,
            "ones1": ones1,
            "onesc": onesc,
        })
    return in_maps


def run_spmd(in_maps):
    """Execute on devices; cached-jit fast path with library fallback."""
    global _compiled, _runner
    if _compiled is None:
        _compiled = _build_program()
    if _runner is None:
        try:
            _runner = _make_runner(_compiled, len(in_maps))
        except Exception:
            _runner = False
    if _runner:
        try:
            return _runner(in_maps)
        except Exception:
            pass
    res = run_bass_kernel_spmd(_compiled, in_maps, list(range(len(in_maps))))
    return res.results


def kernel(foreground, mask, _results_hook=None):
    foreground = np.asarray(foreground, np.float32)
    mask = np.asarray(mask, np.float32)
    B = foreground.shape[0]

    in_maps = make_in_maps(foreground, mask)
    results = run_spmd(in_maps)

    out = np.empty_like(foreground)
    for s in range(B):
        rec = np.asarray(results[s]["out"]).astype(np.float32)
        m = mask[s]
        out[s] = rec * m / 9.0 + foreground[s] * (1.0 - m)
    return out


# revision 7
# speedup vs baseline: 85.0922x; 3.8916x over previous
"""Contextual-attention kernel for Trainium2 — transfer-minimal version.

The axon tunnel makes host<->device bytes the dominant cost (~235ms call
floor, low effective MB/s). So: upload ONLY fp16 foreground (1MB) +
fp16 mask (8KB) per sample, one sample per core (4 cores), compute all
operand prep on device (background slab, patch norms, 3x3-box G maps,
patch transposes via PE), run scores->softmax->reconstruction, do
col2im on device, download fp16 rec [128,64,64] (1MB). Host does only
the final rec*m/9 + fg*(1-m) combine in f32.

Math (validated against the jax reference):
  scores[l,p] = rn[l] * sum_kk V[kk,l] * Gbox[kk,p]      (matmul1)
  E = exp(scores - max_l scores)
  out9[(di,dj)][c,p] = sum_l (rn[l]*V[l,(di,dj,c)]) * E[l,p]   (matmul2)
  Z[p] = sum_l E[l,p];  rec = col2im(out9 / Z)
"""
import sys
for _p in ('/opt/trn_rl_repo',):
    if _p not in sys.path:
        sys.path.insert(0, _p)

import numpy as np

import concourse.bass as bass
import concourse.mybir as mybir
import concourse.tile as tile
from concourse import bacc
from concourse.bass_utils import run_bass_kernel_spmd

EPS = 1e-7
C, H, W = 128, 64, 64
L = H * W                      # 4096
CW = 512                       # pixel-chunk width (8 image rows)
NCHUNK = L // CW               # 8 chunks (full sample per core)
LT = 32                        # l-tiles of 128
F32 = mybir.dt.float32
F16 = mybir.dt.float16
AX = mybir.AxisListType.XYZW
OP = mybir.AluOpType

_compiled = None
_runner = None


def _build_program():
    nc = bacc.Bacc("TRN2", target_bir_lowering=False, debug=False)
    fg16_d = nc.dram_tensor("fg16", [C, H, W], F16, kind="ExternalInput").ap()
    m16_d = nc.dram_tensor("m16", [1, H, W], F16, kind="ExternalInput").ap()
    ident_d = nc.dram_tensor("ident", [C, C], F32, kind="ExternalInput").ap()
    ones1_d = nc.dram_tensor("ones1", [1, C], F32, kind="ExternalInput").ap()
    onesc_d = nc.dram_tensor("onesc", [C, 1], F32, kind="ExternalInput").ap()
    out_d = nc.dram_tensor("out", [C, H, W], F16, kind="ExternalOutput").ap()

    with tile.TileContext(nc) as tc:
        with (
            tc.tile_pool(name="const", bufs=1) as cpool,
            tc.tile_pool(name="pers", bufs=1) as pers,
            tc.tile_pool(name="dram", bufs=1, space="DRAM") as dpool,
            tc.tile_pool(name="pa", bufs=2, space="PSUM") as pa,
            tc.tile_pool(name="pb", bufs=5, space="PSUM") as pb,
            tc.tile_pool(name="pz", bufs=1, space="PSUM") as pz,
        ):
            ident = cpool.tile([C, C], F32)
            nc.sync.dma_start(out=ident[:], in_=ident_d[:])
            ones1 = cpool.tile([1, C], F32)
            nc.sync.dma_start(out=ones1[:], in_=ones1_d[:])
            onesc = cpool.tile([C, 1], F32)
            nc.sync.dma_start(out=onesc[:], in_=onesc_d[:])

            slab3 = pers.tile([C, 3, H + 2, W], F32)   # pad(bg)+EPS, x-shifted
            rnt = pers.tile([C, LT], F32)              # 1/||patch||
            rec = pers.tile([C, H, W], F32)            # col2im accumulator
            nc.vector.memset(rec[:], 0.0)

            tv_d = dpool.tile([LT, C, 9 * C], F32)     # T: rn-scaled patch rows
            g_d = dpool.tile([NCHUNK, C, 9, 8, W], F32)  # box-summed G chunks

            # ================= prep phases (fg32 scoped) =================
            with tc.tile_pool(name="fgp", bufs=1) as fgp:
                fg16 = fgp.tile([C, H, W], F16)
                nc.sync.dma_start(out=fg16[:], in_=fg16_d[:])
                fg32 = fgp.tile([C, H, W], F32)
                nc.vector.tensor_copy(out=fg32[:], in_=fg16[:])

                # ---- bg = fg*(1-m): broadcast (1-m) over partitions via PE
                with tc.tile_pool(name="ph1", bufs=1) as ph1:
                    m16 = ph1.tile([1, H, W], F16)
                    nc.sync.dma_start(out=m16[:], in_=m16_d[:])
                    om32 = ph1.tile([1, H, W], F32)
                    nc.scalar.activation(om32[:], m16[:],
                                         mybir.ActivationFunctionType.Copy,
                                         bias=1.0, scale=-1.0)
                    omb = ph1.tile([C, H, W], F32)
                    for j in range(8):
                        ps = pa.tile([C, 8, W], F32, tag="pa", name=f"ob_{j}")
                        nc.tensor.matmul(ps[:], ones1[:],
                                         om32[:, 8 * j:8 * (j + 1), :],
                                         start=True, stop=True)
                        nc.vector.tensor_copy(
                            out=omb[:, 8 * j:8 * (j + 1), :], in_=ps[:])
                    bg = ph1.tile([C, H, W], F32)
                    nc.vector.tensor_tensor(out=bg[:], in0=fg32[:], in1=omb[:],
                                            op=OP.mult)

                    # slab3[c,dj,yk,x] = vslab[c,yk,x+dj], vslab=pad(bg)+EPS
                    nc.vector.memset(slab3[:], EPS)
                    nc.vector.tensor_scalar(
                        out=slab3[:, 0, 1:H + 1, 1:64], in0=bg[:, :, 0:63],
                        scalar1=EPS, scalar2=None, op0=OP.add)
                    nc.vector.tensor_scalar(
                        out=slab3[:, 1, 1:H + 1, :], in0=bg[:, :, :],
                        scalar1=EPS, scalar2=None, op0=OP.add)
                    nc.vector.tensor_scalar(
                        out=slab3[:, 2, 1:H + 1, 0:63], in0=bg[:, :, 1:64],
                        scalar1=EPS, scalar2=None, op0=OP.add)

                # ---- patch norms -> rnt
                with tc.tile_pool(name="ph2", bufs=1) as ph2:
                    sq3 = ph2.tile([C, 3, H + 2, W], F32)
                    nc.scalar.square(out=sq3[:], in_=slab3[:])
                    acc = ph2.tile([C, H, W], F32)
                    nc.vector.tensor_copy(out=acc[:], in_=sq3[:, 0, 0:H, :])
                    for dj in range(3):
                        for di in range(3):
                            if dj == 0 and di == 0:
                                continue
                            nc.vector.tensor_tensor(
                                out=acc[:], in0=acc[:],
                                in1=sq3[:, dj, di:di + H, :], op=OP.add)
                    nrm = ph2.tile([C, LT], F32)
                    for lt in range(LT):
                        tps = pa.tile([C, C], F32, tag="pa", name=f"nt_{lt}")
                        nc.tensor.transpose(
                            tps[:], acc[:, 2 * lt:2 * lt + 2, :], ident[:])
                        tsb = ph2.tile([C, C], F32, tag="ntb")
                        nc.vector.tensor_copy(out=tsb[:], in_=tps[:])
                        nc.vector.tensor_reduce(nrm[:, lt:lt + 1], tsb[:],
                                                axis=AX, op=OP.add)
                    nc.scalar.sqrt(out=nrm[:], in_=nrm[:])
                    nc.vector.reciprocal(out=rnt[:], in_=nrm[:])

                # ---- T tiles: rn * transpose(slab patch rows) -> DRAM
                with tc.tile_pool(name="ph3", bufs=4) as ph3:
                    for lt in range(LT):
                        for k in range(9):
                            di, dj = k // 3, k % 3
                            tps = pa.tile([C, C], F32, tag="pa",
                                          name=f"tt_{lt}_{k}")
                            nc.tensor.transpose(
                                tps[:],
                                slab3[:, dj, 2 * lt + di:2 * lt + di + 2, :],
                                ident[:])
                            tsb = ph3.tile([C, C], F32, tag="tsb")
                            nc.vector.tensor_scalar(
                                out=tsb[:], in0=tps[:],
                                scalar1=rnt[:, lt:lt + 1], scalar2=None,
                                op0=OP.mult)
                            nc.sync.dma_start(
                                out=tv_d[lt, :, k * C:(k + 1) * C], in_=tsb[:])

                # ---- G: H-pass canvases, per-chunk V-pass -> DRAM
                with tc.tile_pool(name="ph4", bufs=1) as ph4:
                    hcv = ph4.tile([C, 3, H + 2, W], F32)
                    nc.vector.memset(hcv[:], 0.0)
                    sdj = ph4.tile([C, H, W], F32, tag="sdj")
                    for dj in range(3):
                        if dj == 1:
                            src = fg32
                        else:
                            src = sdj
                            if dj == 0:
                                nc.vector.memset(sdj[:, :, 0:1], 0.0)
                                nc.vector.tensor_copy(out=sdj[:, :, 1:64],
                                                      in_=fg32[:, :, 0:63])
                            else:
                                nc.vector.memset(sdj[:, :, 63:64], 0.0)
                                nc.vector.tensor_copy(out=sdj[:, :, 0:63],
                                                      in_=fg32[:, :, 1:64])
                        # H-taps (x-position clipped) into canvas rows 1..64
                        nc.vector.tensor_copy(out=hcv[:, dj, 1:H + 1, :],
                                              in_=src[:])
                        nc.vector.tensor_tensor(
                            out=hcv[:, dj, 1:H + 1, 1:64],
                            in0=hcv[:, dj, 1:H + 1, 1:64],
                            in1=src[:, :, 0:63], op=OP.add)
                        nc.vector.tensor_tensor(
                            out=hcv[:, dj, 1:H + 1, 0:63],
                            in0=hcv[:, dj, 1:H + 1, 0:63],
                            in1=src[:, :, 1:64], op=OP.add)
                    with tc.tile_pool(name="gph", bufs=1) as gph:
                        for ch in range(NCHUNK):
                            r0 = 8 * ch
                            gt = gph.tile([C, 9, 8, W], F32, tag="gt")
                            for k in range(9):
                                di, dj = k // 3, k % 3
                                nc.vector.tensor_copy(
                                    out=gt[:, k, :, :],
                                    in_=hcv[:, dj, r0 + di:r0 + di + 8, :])
                                for u in (-1, 1):
                                    dylo = max(0, -u - r0)
                                    dyhi = min(7, 63 - u - r0)
                                    if dylo > dyhi:
                                        continue
                                    nc.vector.tensor_tensor(
                                        out=gt[:, k, dylo:dyhi + 1, :],
                                        in0=gt[:, k, dylo:dyhi + 1, :],
                                        in1=hcv[:, dj,
                                                r0 + dylo + u + di:
                                                r0 + dyhi + u + di + 1, :],
                                        op=OP.add)
                            nc.sync.dma_start(out=g_d[ch, :, :, :, :],
                                              in_=gt[:])

            # ================= main loop over pixel chunks =================
            with (
                tc.tile_pool(name="sspool", bufs=1) as sspool,
                tc.tile_pool(name="gmain", bufs=1) as gmain,
                tc.tile_pool(name="vpool", bufs=3) as vpool,
                tc.tile_pool(name="small", bufs=2) as small,
                tc.tile_pool(name="mo", bufs=4) as mopool,
            ):
                for ch in range(NCHUNK):
                    r0 = 8 * ch
                    gt = gmain.tile([C, 9, 8, W], F32, tag="gt")
                    nc.sync.dma_start(out=gt[:], in_=g_d[ch, :, :, :, :])

                    # matmul1: scores for all 32 l-tiles at this chunk
                    ss = sspool.tile([C, LT, CW], F32, tag="ss")
                    for lt in range(LT):
                        ps = pa.tile([C, CW], F32, tag="pa",
                                     name=f"m1_{ch}_{lt}")
                        for k in range(9):
                            di, dj = k // 3, k % 3
                            lhsT = slab3[:, dj, 2 * lt + di:2 * lt + di + 2, :]
                            nc.tensor.matmul(ps[:], lhsT, gt[:, k, :, :],
                                             start=(k == 0), stop=(k == 8))
                        nc.vector.tensor_scalar(
                            out=ss[:, lt, :], in0=ps[:],
                            scalar1=rnt[:, lt:lt + 1], scalar2=None,
                            op0=OP.mult)

                    # max over l: 32 tiles then across partitions via PE
                    mrun = small.tile([C, CW], F32, tag="mrun")
                    nc.vector.tensor_copy(out=mrun[:], in_=ss[:, 0, :])
                    for lt in range(1, LT):
                        nc.vector.tensor_tensor(out=mrun[:], in0=mrun[:],
                                                in1=ss[:, lt, :], op=OP.max)
                    mb = small.tile([C, CW], F32, tag="mb", name=f"mb_{ch}")
                    for b in range(4):
                        tps = pa.tile([C, C], F32, tag="pa",
                                      name=f"tp_{ch}_{b}")
                        nc.tensor.transpose(tps[:], mrun[:, b * C:(b + 1) * C],
                                            ident[:])
                        tms = small.tile([C, C], F32, tag="tms",
                                         name=f"tms_{ch}_{b}")
                        nc.vector.tensor_copy(out=tms[:], in_=tps[:])
                        mcol = small.tile([C, 1], F32, tag="mcol",
                                          name=f"mc_{ch}_{b}")
                        nc.vector.tensor_reduce(mcol[:], tms[:], axis=AX,
                                                op=OP.max)
                        tp2 = pa.tile([1, C], F32, tag="pa",
                                      name=f"t2_{ch}_{b}")
                        nc.tensor.transpose(tp2[:], mcol[:], ident[:])
                        mrow = small.tile([1, C], F32, tag="mrow",
                                          name=f"mr_{ch}_{b}")
                        nc.vector.tensor_copy(out=mrow[:], in_=tp2[:])
                        bps = pa.tile([C, C], F32, tag="pa",
                                      name=f"bp_{ch}_{b}")
                        nc.tensor.matmul(bps[:], ones1[:], mrow[:],
                                         start=True, stop=True)
                        nc.vector.tensor_copy(out=mb[:, b * C:(b + 1) * C],
                                              in_=bps[:])

                    # E = exp(ss - mb)
                    for lt in range(LT):
                        sl = ss[:, lt, :]
                        nc.vector.tensor_tensor(out=sl, in0=sl, in1=mb[:],
                                                op=OP.subtract)
                        nc.scalar.activation(sl, sl,
                                             mybir.ActivationFunctionType.Exp)

                    # matmul2: out9 planes [c,p] + Z, two PSUM passes
                    mos = []
                    zrow = pz.tile([1, CW], F32, tag="z", name=f"z_{ch}")
                    rzb = None
                    for pass_i, planes in enumerate(((0, 1, 2, 3, 4),
                                                     (5, 6, 7, 8))):
                        c0 = planes[0] * C
                        c1 = (planes[-1] + 1) * C
                        pts = [pb.tile([C, 8, W], F32, tag="o",
                                       name=f"o_{ch}_{j}") for j in planes]
                        for ls in range(LT):
                            vb = vpool.tile([C, 5 * C], F32, tag="vb")
                            nc.sync.dma_start(out=vb[:, :c1 - c0],
                                              in_=tv_d[ls, :, c0:c1])
                            for i, j in enumerate(planes):
                                nc.tensor.matmul(
                                    pts[i][:], vb[:, i * C:(i + 1) * C],
                                    ss[:, ls, :],
                                    start=(ls == 0), stop=(ls == LT - 1))
                            if pass_i == 0:
                                nc.tensor.matmul(zrow[:], onesc[:],
                                                 ss[:, ls, :],
                                                 start=(ls == 0),
                                                 stop=(ls == LT - 1))
                        if pass_i == 0:
                            zs = small.tile([1, CW], F32, tag="zs",
                                            name=f"zs_{ch}")
                            nc.vector.tensor_copy(out=zs[:], in_=zrow[:])
                            rz = small.tile([1, CW], F32, tag="rz",
                                            name=f"rz_{ch}")
                            nc.vector.reciprocal(out=rz[:], in_=zs[:])
                            bps = pa.tile([C, 8, W], F32, tag="pa",
                                          name=f"zb_{ch}")
                            nc.tensor.matmul(bps[:], ones1[:], rz[:],
                                             start=True, stop=True)
                            rzb = small.tile([C, 8, W], F32, tag="rzb",
                                             name=f"rzb_{ch}")
                            nc.vector.tensor_copy(out=rzb[:], in_=bps[:])
                        for i, j in enumerate(planes):
                            mo = mopool.tile([C, 8, W], F32, tag="mo",
                                             name=f"mo_{ch}_{j}")
                            nc.vector.tensor_tensor(out=mo[:], in0=pts[i][:],
                                                    in1=rzb[:], op=OP.mult)
                            mos.append((j, mo))

                    # col2im: rec[y,x] += mo[(di,dj)][y+1-di, x+1-dj]
                    for j, mo in mos:
                        di, dj = j // 3, j % 3
                        ylo = max(0, r0 + di - 1)
                        yhi = min(63, r0 + di + 6)
                        if ylo > yhi:
                            continue
                        sy = ylo - (r0 + di - 1)
                        nr = yhi - ylo + 1
                        if dj == 0:
                            xd, xs, ncol = 0, 1, 63
                        elif dj == 1:
                            xd, xs, ncol = 0, 0, 64
                        else:
                            xd, xs, ncol = 1, 0, 63
                        nc.vector.tensor_tensor(
                            out=rec[:, ylo:ylo + nr, xd:xd + ncol],
                            in0=rec[:, ylo:ylo + nr, xd:xd + ncol],
                            in1=mo[:, sy:sy + nr, xs:xs + ncol],
                            op=OP.add)

                # ---- fp16 cast + output
                out16 = small.tile([C, H, W], F16, tag="o16")
                nc.vector.tensor_copy(out=out16[:], in_=rec[:])
                nc.sync.dma_start(out=out_d[:], in_=out16[:])
    nc.compile()
    return nc


def _make_runner(nc, n_cores):
    """Cached-jit SPMD executor: builds the shard_map wrapper ONCE.

    run_bass_kernel_spmd re-traces + re-jits per call (~650ms overhead under
    axon); this path also creates the donated zero output buffers on device
    (sharded jnp.zeros) instead of uploading them from host each call.
    """
    import jax
    import jax.numpy as jnp
    from jax.sharding import Mesh, PartitionSpec, NamedSharding
    import warnings
    with warnings.catch_warnings():
        warnings.simplefilter("ignore")
        try:
            from jax.experimental.shard_map import shard_map
        except ImportError:
            from jax import shard_map as _sm

            def shard_map(f, **kw):
                kw["check_vma"] = kw.pop("check_rep", False)
                return _sm(f, **kw)
    from concourse.bass2jax import (_bass_exec_p, install_neuronx_cc_hook,
                                    partition_id_tensor)

    install_neuronx_cc_hook()
    partition_name = nc.partition_id_tensor.name if nc.partition_id_tensor else None
    in_names, out_names, out_avals = [], [], []
    for alloc in nc.m.functions[0].allocations:
        if not isinstance(alloc, mybir.MemoryLocationSet):
            continue
        name = alloc.memorylocations[0].name
        if alloc.kind == "ExternalInput":
            if name != partition_name:
                in_names.append(name)
        elif alloc.kind == "ExternalOutput":
            out_names.append(name)
            out_avals.append(jax.core.ShapedArray(
                tuple(alloc.tensor_shape), mybir.dt.np(alloc.dtype)))
    n_params = len(in_names)
    n_outs = len(out_avals)
    all_in_names = list(in_names) + list(out_names)
    if partition_name is not None:
        all_in_names.append(partition_name)

    def _body(*args):
        operands = list(args)
        if partition_name is not None:
            operands.append(partition_id_tensor())
        return tuple(_bass_exec_p.bind(
            *operands, out_avals=tuple(out_avals), in_names=tuple(all_in_names),
            out_names=tuple(out_names), lowering_input_output_aliases=(),
            sim_require_finite=True, sim_require_nnan=True, nc=nc))

    devices = jax.devices()[:n_cores]
    mesh = Mesh(np.asarray(devices), ("core",))
    in_specs = (PartitionSpec("core"),) * (n_params + n_outs)
    out_specs = (PartitionSpec("core"),) * n_outs
    donate = tuple(range(n_params, n_params + n_outs))
    sharded = jax.jit(
        shard_map(_body, mesh=mesh, in_specs=in_specs, out_specs=out_specs,
                  check_rep=False),
        donate_argnums=donate, keep_unused=True)

    zero_shardings = [NamedSharding(mesh, PartitionSpec("core"))] * n_outs

    def _zeros():
        return tuple(jnp.zeros((n_cores * a.shape[0], *a.shape[1:]), a.dtype)
                     for a in out_avals)

    make_zeros = jax.jit(_zeros, out_shardings=tuple(zero_shardings))

    def run(in_maps):
        concat_in = [
            np.concatenate([np.asarray(in_maps[c][nm]) for c in range(n_cores)],
                           axis=0) for nm in in_names]
        out_arrs = sharded(*concat_in, *make_zeros())
        out_arrs = [np.asarray(a) for a in out_arrs]
        return [{nm: out_arrs[i].reshape(n_cores, *out_avals[i].shape)[c]
                 for i, nm in enumerate(out_names)} for c in range(n_cores)]

    return run


def make_in_maps(foreground, mask):
    foreground = np.asarray(foreground, np.float32)
    mask = np.asarray(mask, np.float32)
    ident = np.eye(C, dtype=np.float32)
    ones1 = np.ones((1, C), np.float32)
    onesc = np.ones((C, 1), np.float32)
    in_maps = []
    for s in range(foreground.shape[0]):
        in_maps.append({
            "fg16": foreground[s].astype(np.float16),
            "m16": mask[s].astype(np.float16),
            "ident": ident,
            "ones1": ones1,
            "onesc": onesc,
        })
    return in_maps


def run_spmd(in_maps):
    """Execute on devices; cached-jit fast path with library fallback."""
    global _compiled, _runner
    if _compiled is None:
        _compiled = _build_program()
    if _runner is None:
        try:
            _runner = _make_runner(_compiled, len(in_maps))
        except Exception:
            _runner = False
    if _runner:
        try:
            return _runner(in_maps)
        except Exception:
            pass
    res = run_bass_kernel_spmd(_compiled, in_maps, list(range(len(in_maps))))
    return res.results


def kernel(foreground, mask, _results_hook=None):
    foreground = np.asarray(foreground, np.float32)
    mask = np.asarray(mask, np.float32)
    B = foreground.shape[0]

    in_maps = make_in_maps(foreground, mask)
    results = run_spmd(in_maps)

    out = np.empty_like(foreground)
    for s in range(B):
        rec = np.asarray(results[s]["out"]).astype(np.float32)
        m = mask[s]
        out[s] = rec * m / 9.0 + foreground[s] * (1.0 - m)
    return out


# revision 8
# speedup vs baseline: 108.0667x; 1.2700x over previous
"""Contextual-attention kernel for Trainium2 — transfer-minimal version.

The axon tunnel makes host<->device bytes the dominant cost (~235ms call
floor, low effective MB/s). So: upload ONLY fp16 foreground (1MB) +
fp16 mask (8KB) per sample, one sample per core (4 cores), compute all
operand prep on device (background slab, patch norms, 3x3-box G maps,
patch transposes via PE), run scores->softmax->reconstruction, do
col2im on device, download fp16 rec [128,64,64] (1MB). Host does only
the final rec*m/9 + fg*(1-m) combine in f32.

Math (validated against the jax reference):
  scores[l,p] = rn[l] * sum_kk V[kk,l] * Gbox[kk,p]      (matmul1)
  E = exp(scores - max_l scores)
  out9[(di,dj)][c,p] = sum_l (rn[l]*V[l,(di,dj,c)]) * E[l,p]   (matmul2)
  Z[p] = sum_l E[l,p];  rec = col2im(out9 / Z)
"""
import sys
for _p in ('/opt/trn_rl_repo',):
    if _p not in sys.path:
        sys.path.insert(0, _p)

import numpy as np

import concourse.bass as bass
import concourse.mybir as mybir
import concourse.tile as tile
from concourse import bacc
from concourse.bass_utils import run_bass_kernel_spmd

EPS = 1e-7
C, H, W = 128, 64, 64
L = H * W                      # 4096
CW = 512                       # pixel-chunk width (8 image rows)
NCHUNK = L // CW               # 8 chunks (full sample per core)
LT = 32                        # l-tiles of 128
F32 = mybir.dt.float32
F16 = mybir.dt.float16
F8 = mybir.dt.float8e4
AX = mybir.AxisListType.XYZW
OP = mybir.AluOpType

_compiled = None
_runner = None


def _build_program():
    nc = bacc.Bacc("TRN2", target_bir_lowering=False, debug=False)
    fg16_d = nc.dram_tensor("fg16", [C, H, W], F8, kind="ExternalInput").ap()
    m16_d = nc.dram_tensor("m16", [1, H, W], F16, kind="ExternalInput").ap()
    ident_d = nc.dram_tensor("ident", [C, C], F32, kind="ExternalInput").ap()
    ones1_d = nc.dram_tensor("ones1", [1, C], F32, kind="ExternalInput").ap()
    onesc_d = nc.dram_tensor("onesc", [C, 1], F32, kind="ExternalInput").ap()
    out_d = nc.dram_tensor("out", [C, H, W], F16, kind="ExternalOutput").ap()

    with tile.TileContext(nc) as tc:
        with (
            tc.tile_pool(name="const", bufs=1) as cpool,
            tc.tile_pool(name="pers", bufs=1) as pers,
            tc.tile_pool(name="dram", bufs=1, space="DRAM") as dpool,
            tc.tile_pool(name="pa", bufs=2, space="PSUM") as pa,
            tc.tile_pool(name="pb", bufs=5, space="PSUM") as pb,
            tc.tile_pool(name="pz", bufs=1, space="PSUM") as pz,
        ):
            ident = cpool.tile([C, C], F32)
            nc.sync.dma_start(out=ident[:], in_=ident_d[:])
            ones1 = cpool.tile([1, C], F32)
            nc.sync.dma_start(out=ones1[:], in_=ones1_d[:])
            onesc = cpool.tile([C, 1], F32)
            nc.sync.dma_start(out=onesc[:], in_=onesc_d[:])

            slab3 = pers.tile([C, 3, H + 2, W], F32)   # pad(bg)+EPS, x-shifted
            rnt = pers.tile([C, LT], F32)              # 1/||patch||
            rec = pers.tile([C, H, W], F32)            # col2im accumulator
            nc.vector.memset(rec[:], 0.0)

            tv_d = dpool.tile([LT, C, 9 * C], F32)     # T: rn-scaled patch rows
            g_d = dpool.tile([NCHUNK, C, 9, 8, W], F32)  # box-summed G chunks

            # ================= prep phases (fg32 scoped) =================
            with tc.tile_pool(name="fgp", bufs=1) as fgp:
                fg16 = fgp.tile([C, H, W], F8)
                nc.sync.dma_start(out=fg16[:], in_=fg16_d[:])
                fg32 = fgp.tile([C, H, W], F32)
                nc.vector.tensor_copy(out=fg32[:], in_=fg16[:])

                # ---- bg = fg*(1-m): broadcast (1-m) over partitions via PE
                with tc.tile_pool(name="ph1", bufs=1) as ph1:
                    m16 = ph1.tile([1, H, W], F16)
                    nc.sync.dma_start(out=m16[:], in_=m16_d[:])
                    om32 = ph1.tile([1, H, W], F32)
                    nc.scalar.activation(om32[:], m16[:],
                                         mybir.ActivationFunctionType.Copy,
                                         bias=1.0, scale=-1.0)
                    omb = ph1.tile([C, H, W], F32)
                    for j in range(8):
                        ps = pa.tile([C, 8, W], F32, tag="pa", name=f"ob_{j}")
                        nc.tensor.matmul(ps[:], ones1[:],
                                         om32[:, 8 * j:8 * (j + 1), :],
                                         start=True, stop=True)
                        nc.vector.tensor_copy(
                            out=omb[:, 8 * j:8 * (j + 1), :], in_=ps[:])
                    bg = ph1.tile([C, H, W], F32)
                    nc.vector.tensor_tensor(out=bg[:], in0=fg32[:], in1=omb[:],
                                            op=OP.mult)

                    # slab3[c,dj,yk,x] = vslab[c,yk,x+dj], vslab=pad(bg)+EPS
                    nc.vector.memset(slab3[:], EPS)
                    nc.vector.tensor_scalar(
                        out=slab3[:, 0, 1:H + 1, 1:64], in0=bg[:, :, 0:63],
                        scalar1=EPS, scalar2=None, op0=OP.add)
                    nc.vector.tensor_scalar(
                        out=slab3[:, 1, 1:H + 1, :], in0=bg[:, :, :],
                        scalar1=EPS, scalar2=None, op0=OP.add)
                    nc.vector.tensor_scalar(
                        out=slab3[:, 2, 1:H + 1, 0:63], in0=bg[:, :, 1:64],
                        scalar1=EPS, scalar2=None, op0=OP.add)

                # ---- patch norms -> rnt
                with tc.tile_pool(name="ph2", bufs=1) as ph2:
                    sq3 = ph2.tile([C, 3, H + 2, W], F32)
                    nc.scalar.square(out=sq3[:], in_=slab3[:])
                    acc = ph2.tile([C, H, W], F32)
                    nc.vector.tensor_copy(out=acc[:], in_=sq3[:, 0, 0:H, :])
                    for dj in range(3):
                        for di in range(3):
                            if dj == 0 and di == 0:
                                continue
                            nc.vector.tensor_tensor(
                                out=acc[:], in0=acc[:],
                                in1=sq3[:, dj, di:di + H, :], op=OP.add)
                    nrm = ph2.tile([C, LT], F32)
                    for lt in range(LT):
                        tps = pa.tile([C, C], F32, tag="pa", name=f"nt_{lt}")
                        nc.tensor.transpose(
                            tps[:], acc[:, 2 * lt:2 * lt + 2, :], ident[:])
                        tsb = ph2.tile([C, C], F32, tag="ntb")
                        nc.vector.tensor_copy(out=tsb[:], in_=tps[:])
                        nc.vector.tensor_reduce(nrm[:, lt:lt + 1], tsb[:],
                                                axis=AX, op=OP.add)
                    nc.scalar.sqrt(out=nrm[:], in_=nrm[:])
                    nc.vector.reciprocal(out=rnt[:], in_=nrm[:])

                # ---- T tiles: rn * transpose(slab patch rows) -> DRAM
                with tc.tile_pool(name="ph3", bufs=4) as ph3:
                    for lt in range(LT):
                        for k in range(9):
                            di, dj = k // 3, k % 3
                            tps = pa.tile([C, C], F32, tag="pa",
                                          name=f"tt_{lt}_{k}")
                            nc.tensor.transpose(
                                tps[:],
                                slab3[:, dj, 2 * lt + di:2 * lt + di + 2, :],
                                ident[:])
                            tsb = ph3.tile([C, C], F32, tag="tsb")
                            nc.vector.tensor_scalar(
                                out=tsb[:], in0=tps[:],
                                scalar1=rnt[:, lt:lt + 1], scalar2=None,
                                op0=OP.mult)
                            nc.sync.dma_start(
                                out=tv_d[lt, :, k * C:(k + 1) * C], in_=tsb[:])

                # ---- G: H-pass canvases, per-chunk V-pass -> DRAM
                with tc.tile_pool(name="ph4", bufs=1) as ph4:
                    hcv = ph4.tile([C, 3, H + 2, W], F32)
                    nc.vector.memset(hcv[:], 0.0)
                    sdj = ph4.tile([C, H, W], F32, tag="sdj")
                    for dj in range(3):
                        if dj == 1:
                            src = fg32
                        else:
                            src = sdj
                            if dj == 0:
                                nc.vector.memset(sdj[:, :, 0:1], 0.0)
                                nc.vector.tensor_copy(out=sdj[:, :, 1:64],
                                                      in_=fg32[:, :, 0:63])
                            else:
                                nc.vector.memset(sdj[:, :, 63:64], 0.0)
                                nc.vector.tensor_copy(out=sdj[:, :, 0:63],
                                                      in_=fg32[:, :, 1:64])
                        # H-taps (x-position clipped) into canvas rows 1..64
                        nc.vector.tensor_copy(out=hcv[:, dj, 1:H + 1, :],
                                              in_=src[:])
                        nc.vector.tensor_tensor(
                            out=hcv[:, dj, 1:H + 1, 1:64],
                            in0=hcv[:, dj, 1:H + 1, 1:64],
                            in1=src[:, :, 0:63], op=OP.add)
                        nc.vector.tensor_tensor(
                            out=hcv[:, dj, 1:H + 1, 0:63],
                            in0=hcv[:, dj, 1:H + 1, 0:63],
                            in1=src[:, :, 1:64], op=OP.add)
                    with tc.tile_pool(name="gph", bufs=1) as gph:
                        for ch in range(NCHUNK):
                            r0 = 8 * ch
                            gt = gph.tile([C, 9, 8, W], F32, tag="gt")
                            for k in range(9):
                                di, dj = k // 3, k % 3
                                nc.vector.tensor_copy(
                                    out=gt[:, k, :, :],
                                    in_=hcv[:, dj, r0 + di:r0 + di + 8, :])
                                for u in (-1, 1):
                                    dylo = max(0, -u - r0)
                                    dyhi = min(7, 63 - u - r0)
                                    if dylo > dyhi:
                                        continue
                                    nc.vector.tensor_tensor(
                                        out=gt[:, k, dylo:dyhi + 1, :],
                                        in0=gt[:, k, dylo:dyhi + 1, :],
                                        in1=hcv[:, dj,
                                                r0 + dylo + u + di:
                                                r0 + dyhi + u + di + 1, :],
                                        op=OP.add)
                            nc.sync.dma_start(out=g_d[ch, :, :, :, :],
                                              in_=gt[:])

            # ================= main loop over pixel chunks =================
            with (
                tc.tile_pool(name="sspool", bufs=1) as sspool,
                tc.tile_pool(name="gmain", bufs=1) as gmain,
                tc.tile_pool(name="vpool", bufs=3) as vpool,
                tc.tile_pool(name="small", bufs=2) as small,
                tc.tile_pool(name="mo", bufs=4) as mopool,
            ):
                for ch in range(NCHUNK):
                    r0 = 8 * ch
                    gt = gmain.tile([C, 9, 8, W], F32, tag="gt")
                    nc.sync.dma_start(out=gt[:], in_=g_d[ch, :, :, :, :])

                    # matmul1: scores for all 32 l-tiles at this chunk
                    ss = sspool.tile([C, LT, CW], F32, tag="ss")
                    for lt in range(LT):
                        ps = pa.tile([C, CW], F32, tag="pa",
                                     name=f"m1_{ch}_{lt}")
                        for k in range(9):
                            di, dj = k // 3, k % 3
                            lhsT = slab3[:, dj, 2 * lt + di:2 * lt + di + 2, :]
                            nc.tensor.matmul(ps[:], lhsT, gt[:, k, :, :],
                                             start=(k == 0), stop=(k == 8))
                        nc.vector.tensor_scalar(
                            out=ss[:, lt, :], in0=ps[:],
                            scalar1=rnt[:, lt:lt + 1], scalar2=None,
                            op0=OP.mult)

                    # max over l: 32 tiles then across partitions via PE
                    mrun = small.tile([C, CW], F32, tag="mrun")
                    nc.vector.tensor_copy(out=mrun[:], in_=ss[:, 0, :])
                    for lt in range(1, LT):
                        nc.vector.tensor_tensor(out=mrun[:], in0=mrun[:],
                                                in1=ss[:, lt, :], op=OP.max)
                    mb = small.tile([C, CW], F32, tag="mb", name=f"mb_{ch}")
                    for b in range(4):
                        tps = pa.tile([C, C], F32, tag="pa",
                                      name=f"tp_{ch}_{b}")
                        nc.tensor.transpose(tps[:], mrun[:, b * C:(b + 1) * C],
                                            ident[:])
                        tms = small.tile([C, C], F32, tag="tms",
                                         name=f"tms_{ch}_{b}")
                        nc.vector.tensor_copy(out=tms[:], in_=tps[:])
                        mcol = small.tile([C, 1], F32, tag="mcol",
                                          name=f"mc_{ch}_{b}")
                        nc.vector.tensor_reduce(mcol[:], tms[:], axis=AX,
                                                op=OP.max)
                        tp2 = pa.tile([1, C], F32, tag="pa",
                                      name=f"t2_{ch}_{b}")
                        nc.tensor.transpose(tp2[:], mcol[:], ident[:])
                        mrow = small.tile([1, C], F32, tag="mrow",
                                          name=f"mr_{ch}_{b}")
                        nc.vector.tensor_copy(out=mrow[:], in_=tp2[:])
                        bps = pa.tile([C, C], F32, tag="pa",
                                      name=f"bp_{ch}_{b}")
                        nc.tensor.matmul(bps[:], ones1[:], mrow[:],
                                         start=True, stop=True)
                        nc.vector.tensor_copy(out=mb[:, b * C:(b + 1) * C],
                                              in_=bps[:])

                    # E = exp(ss - mb)
                    for lt in range(LT):
                        sl = ss[:, lt, :]
                        nc.vector.tensor_tensor(out=sl, in0=sl, in1=mb[:],
                                                op=OP.subtract)
                        nc.scalar.activation(sl, sl,
                                             mybir.ActivationFunctionType.Exp)

                    # matmul2: out9 planes [c,p] + Z, two PSUM passes
                    mos = []
                    zrow = pz.tile([1, CW], F32, tag="z", name=f"z_{ch}")
                    rzb = None
                    for pass_i, planes in enumerate(((0, 1, 2, 3, 4),
                                                     (5, 6, 7, 8))):
                        c0 = planes[0] * C
                        c1 = (planes[-1] + 1) * C
                        pts = [pb.tile([C, 8, W], F32, tag="o",
                                       name=f"o_{ch}_{j}") for j in planes]
                        for ls in range(LT):
                            vb = vpool.tile([C, 5 * C], F32, tag="vb")
                            nc.sync.dma_start(out=vb[:, :c1 - c0],
                                              in_=tv_d[ls, :, c0:c1])
                            for i, j in enumerate(planes):
                                nc.tensor.matmul(
                                    pts[i][:], vb[:, i * C:(i + 1) * C],
                                    ss[:, ls, :],
                                    start=(ls == 0), stop=(ls == LT - 1))
                            if pass_i == 0:
                                nc.tensor.matmul(zrow[:], onesc[:],
                                                 ss[:, ls, :],
                                                 start=(ls == 0),
                                                 stop=(ls == LT - 1))
                        if pass_i == 0:
                            zs = small.tile([1, CW], F32, tag="zs",
                                            name=f"zs_{ch}")
                            nc.vector.tensor_copy(out=zs[:], in_=zrow[:])
                            rz = small.tile([1, CW], F32, tag="rz",
                                            name=f"rz_{ch}")
                            nc.vector.reciprocal(out=rz[:], in_=zs[:])
                            bps = pa.tile([C, 8, W], F32, tag="pa",
                                          name=f"zb_{ch}")
                            nc.tensor.matmul(bps[:], ones1[:], rz[:],
                                             start=True, stop=True)
                            rzb = small.tile([C, 8, W], F32, tag="rzb",
                                             name=f"rzb_{ch}")
                            nc.vector.tensor_copy(out=rzb[:], in_=bps[:])
                        for i, j in enumerate(planes):
                            mo = mopool.tile([C, 8, W], F32, tag="mo",
                                             name=f"mo_{ch}_{j}")
                            nc.vector.tensor_tensor(out=mo[:], in0=pts[i][:],
                                                    in1=rzb[:], op=OP.mult)
                            mos.append((j, mo))

                    # col2im: rec[y,x] += mo[(di,dj)][y+1-di, x+1-dj]
                    for j, mo in mos:
                        di, dj = j // 3, j % 3
                        ylo = max(0, r0 + di - 1)
                        yhi = min(63, r0 + di + 6)
                        if ylo > yhi:
                            continue
                        sy = ylo - (r0 + di - 1)
                        nr = yhi - ylo + 1
                        if dj == 0:
                            xd, xs, ncol = 0, 1, 63
                        elif dj == 1:
                            xd, xs, ncol = 0, 0, 64
                        else:
                            xd, xs, ncol = 1, 0, 63
                        nc.vector.tensor_tensor(
                            out=rec[:, ylo:ylo + nr, xd:xd + ncol],
                            in0=rec[:, ylo:ylo + nr, xd:xd + ncol],
                            in1=mo[:, sy:sy + nr, xs:xs + ncol],
                            op=OP.add)

                # ---- fp16 cast + output
                out16 = small.tile([C, H, W], F16, tag="o16")
                nc.vector.tensor_copy(out=out16[:], in_=rec[:])
                nc.sync.dma_start(out=out_d[:], in_=out16[:])
    nc.compile()
    return nc


def _make_runner(nc, n_cores):
    """Cached-jit SPMD executor: builds the shard_map wrapper ONCE.

    run_bass_kernel_spmd re-traces + re-jits per call (~650ms overhead under
    axon); this path also creates the donated zero output buffers on device
    (sharded jnp.zeros) instead of uploading them from host each call.
    """
    import jax
    import jax.numpy as jnp
    from jax.sharding import Mesh, PartitionSpec, NamedSharding
    import warnings
    with warnings.catch_warnings():
        warnings.simplefilter("ignore")
        try:
            from jax.experimental.shard_map import shard_map
        except ImportError:
            from jax import shard_map as _sm

            def shard_map(f, **kw):
                kw["check_vma"] = kw.pop("check_rep", False)
                return _sm(f, **kw)
    from concourse.bass2jax import (_bass_exec_p, install_neuronx_cc_hook,
                                    partition_id_tensor)

    install_neuronx_cc_hook()
    partition_name = nc.partition_id_tensor.name if nc.partition_id_tensor else None
    in_names, out_names, out_avals = [], [], []
    for alloc in nc.m.functions[0].allocations:
        if not isinstance(alloc, mybir.MemoryLocationSet):
            continue
        name = alloc.memorylocations[0].name
        if alloc.kind == "ExternalInput":
            if name != partition_name:
                in_names.append(name)
        elif alloc.kind == "ExternalOutput":
            out_names.append(name)
            out_avals.append(jax.core.ShapedArray(
                tuple(alloc.tensor_shape), mybir.dt.np(alloc.dtype)))
    n_params = len(in_names)
    n_outs = len(out_avals)
    all_in_names = list(in_names) + list(out_names)
    if partition_name is not None:
        all_in_names.append(partition_name)

    def _body(*args):
        operands = list(args)
        if partition_name is not None:
            operands.append(partition_id_tensor())
        return tuple(_bass_exec_p.bind(
            *operands, out_avals=tuple(out_avals), in_names=tuple(all_in_names),
            out_names=tuple(out_names), lowering_input_output_aliases=(),
            sim_require_finite=True, sim_require_nnan=True, nc=nc))

    devices = jax.devices()[:n_cores]
    mesh = Mesh(np.asarray(devices), ("core",))
    in_specs = (PartitionSpec("core"),) * (n_params + n_outs)
    out_specs = (PartitionSpec("core"),) * n_outs
    donate = tuple(range(n_params, n_params + n_outs))
    sharded = jax.jit(
        shard_map(_body, mesh=mesh, in_specs=in_specs, out_specs=out_specs,
                  check_rep=False),
        donate_argnums=donate, keep_unused=True)

    zero_shardings = [NamedSharding(mesh, PartitionSpec("core"))] * n_outs

    def _zeros():
        return tuple(jnp.zeros((n_cores * a.shape[0], *a.shape[1:]), a.dtype)
                     for a in out_avals)

    make_zeros = jax.jit(_zeros, out_shardings=tuple(zero_shardings))

    def run(in_maps):
        concat_in = [
            np.concatenate([np.asarray(in_maps[c][nm]) for c in range(n_cores)],
                           axis=0) for nm in in_names]
        out_arrs = sharded(*concat_in, *make_zeros())
        out_arrs = [np.asarray(a) for a in out_arrs]
        return [{nm: out_arrs[i].reshape(n_cores, *out_avals[i].shape)[c]
                 for i, nm in enumerate(out_names)} for c in range(n_cores)]

    return run


def make_in_maps(foreground, mask):
    foreground = np.asarray(foreground, np.float32)
    mask = np.asarray(mask, np.float32)
    ident = np.eye(C, dtype=np.float32)
    ones1 = np.ones((1, C), np.float32)
    onesc = np.ones((C, 1), np.float32)
    in_maps = []
    for s in range(foreground.shape[0]):
        in_maps.append({
            "fg16": foreground[s].astype(mybir.dt.np(F8)),
            "m16": mask[s].astype(np.float16),
            "ident": ident,
            "ones1": ones1,
            "onesc": onesc,
        })
    return in_maps


def run_spmd(in_maps):
    """Execute on devices; cached-jit fast path with library fallback."""
    global _compiled, _runner
    if _compiled is None:
        _compiled = _build_program()
    if _runner is None:
        try:
            _runner = _make_runner(_compiled, len(in_maps))
        except Exception:
            _runner = False
    if _runner:
        try:
            return _runner(in_maps)
        except Exception:
            pass
    res = run_bass_kernel_spmd(_compiled, in_maps, list(range(len(in_maps))))
    return res.results


def kernel(foreground, mask, _results_hook=None):
    foreground = np.asarray(foreground, np.float32)
    mask = np.asarray(mask, np.float32)
    B = foreground.shape[0]

    in_maps = make_in_maps(foreground, mask)
    results = run_spmd(in_maps)

    out = np.empty_like(foreground)
    for s in range(B):
        rec = np.asarray(results[s]["out"]).astype(np.float32)
        m = mask[s]
        out[s] = rec * m / 9.0 + foreground[s] * (1.0 - m)
    return out


# revision 10
# speedup vs baseline: 110.4626x; 1.0222x over previous
"""Contextual-attention kernel for Trainium2 — transfer-minimal version.

The axon tunnel makes host<->device bytes the dominant cost (~87ms
cached-call floor, low effective MB/s). So: upload ONLY fp8e4m3
foreground (0.5MB, empirically 5.6e-4 rel err vs the 2e-2 gate) + fp16
mask (8KB) per sample, one sample per core (4 cores), compute all
operand prep on device (background slab, patch norms, 3x3-box G maps,
patch transposes via PE), run scores->softmax->reconstruction, do
col2im on device, download fp16 rec [128,64,64] (1MB). Host does only
the final rec*m/9 + fg*(1-m) combine in f32. Execution goes through a
cached-jit shard_map wrapper (run_bass_kernel_spmd re-jits per call,
~650ms overhead) with donated zero output buffers created on device.

Math (validated against the jax reference):
  scores[l,p] = rn[l] * sum_kk V[kk,l] * Gbox[kk,p]      (matmul1)
  E = exp(scores - max_l scores)
  out9[(di,dj)][c,p] = sum_l (rn[l]*V[l,(di,dj,c)]) * E[l,p]   (matmul2)
  Z[p] = sum_l E[l,p];  rec = col2im(out9 / Z)
"""
import sys
for _p in ('/opt/trn_rl_repo',):
    if _p not in sys.path:
        sys.path.insert(0, _p)

import numpy as np

import concourse.bass as bass
import concourse.mybir as mybir
import concourse.tile as tile
from concourse import bacc
from concourse.bass_utils import run_bass_kernel_spmd

EPS = 1e-7
C, H, W = 128, 64, 64
L = H * W                      # 4096
CW = 512                       # pixel-chunk width (8 image rows)
NCHUNK = L // CW               # 8 chunks (full sample per core)
LT = 32                        # l-tiles of 128
F32 = mybir.dt.float32
F16 = mybir.dt.float16
F8 = mybir.dt.float8e4
AX = mybir.AxisListType.XYZW
OP = mybir.AluOpType

_compiled = None
_runner = None


def _build_program():
    nc = bacc.Bacc("TRN2", target_bir_lowering=False, debug=False)
    fg16_d = nc.dram_tensor("fg16", [C, H, W], F8, kind="ExternalInput").ap()
    m16_d = nc.dram_tensor("m16", [1, H, W], F16, kind="ExternalInput").ap()
    ident_d = nc.dram_tensor("ident", [C, C], F32, kind="ExternalInput").ap()
    ones1_d = nc.dram_tensor("ones1", [1, C], F32, kind="ExternalInput").ap()
    onesc_d = nc.dram_tensor("onesc", [C, 1], F32, kind="ExternalInput").ap()
    out_d = nc.dram_tensor("out", [C, H, W], F16, kind="ExternalOutput").ap()

    with tile.TileContext(nc) as tc:
        with (
            tc.tile_pool(name="const", bufs=1) as cpool,
            tc.tile_pool(name="pers", bufs=1) as pers,
            tc.tile_pool(name="dram", bufs=1, space="DRAM") as dpool,
            tc.tile_pool(name="pa", bufs=2, space="PSUM") as pa,
            tc.tile_pool(name="pb", bufs=5, space="PSUM") as pb,
            tc.tile_pool(name="pz", bufs=1, space="PSUM") as pz,
        ):
            ident = cpool.tile([C, C], F32)
            nc.sync.dma_start(out=ident[:], in_=ident_d[:])
            ones1 = cpool.tile([1, C], F32)
            nc.sync.dma_start(out=ones1[:], in_=ones1_d[:])
            onesc = cpool.tile([C, 1], F32)
            nc.sync.dma_start(out=onesc[:], in_=onesc_d[:])

            slab3 = pers.tile([C, 3, H + 2, W], F32)   # pad(bg)+EPS, x-shifted
            rnt = pers.tile([C, LT], F32)              # 1/||patch||
            rec = pers.tile([C, H, W], F32)            # col2im accumulator
            nc.vector.memset(rec[:], 0.0)

            tv_d = dpool.tile([LT, C, 9 * C], F32)     # T: rn-scaled patch rows
            g_d = dpool.tile([NCHUNK, C, 9, 8, W], F32)  # box-summed G chunks

            # ================= prep phases (fg32 scoped) =================
            with tc.tile_pool(name="fgp", bufs=1) as fgp:
                fg16 = fgp.tile([C, H, W], F8)
                nc.sync.dma_start(out=fg16[:], in_=fg16_d[:])
                fg32 = fgp.tile([C, H, W], F32)
                nc.vector.tensor_copy(out=fg32[:], in_=fg16[:])

                # ---- bg = fg*(1-m): broadcast (1-m) over partitions via PE
                with tc.tile_pool(name="ph1", bufs=1) as ph1:
                    m16 = ph1.tile([1, H, W], F16)
                    nc.sync.dma_start(out=m16[:], in_=m16_d[:])
                    om32 = ph1.tile([1, H, W], F32)
                    nc.scalar.activation(om32[:], m16[:],
                                         mybir.ActivationFunctionType.Copy,
                                         bias=1.0, scale=-1.0)
                    omb = ph1.tile([C, H, W], F32)
                    for j in range(8):
                        ps = pa.tile([C, 8, W], F32, tag="pa", name=f"ob_{j}")
                        nc.tensor.matmul(ps[:], ones1[:],
                                         om32[:, 8 * j:8 * (j + 1), :],
                                         start=True, stop=True)
                        nc.vector.tensor_copy(
                            out=omb[:, 8 * j:8 * (j + 1), :], in_=ps[:])
                    bg = ph1.tile([C, H, W], F32)
                    nc.vector.tensor_tensor(out=bg[:], in0=fg32[:], in1=omb[:],
                                            op=OP.mult)

                    # slab3[c,dj,yk,x] = vslab[c,yk,x+dj], vslab=pad(bg)+EPS
                    nc.vector.memset(slab3[:], EPS)
                    nc.vector.tensor_scalar(
                        out=slab3[:, 0, 1:H + 1, 1:64], in0=bg[:, :, 0:63],
                        scalar1=EPS, scalar2=None, op0=OP.add)
                    nc.vector.tensor_scalar(
                        out=slab3[:, 1, 1:H + 1, :], in0=bg[:, :, :],
                        scalar1=EPS, scalar2=None, op0=OP.add)
                    nc.vector.tensor_scalar(
                        out=slab3[:, 2, 1:H + 1, 0:63], in0=bg[:, :, 1:64],
                        scalar1=EPS, scalar2=None, op0=OP.add)

                # ---- patch norms -> rnt
                with tc.tile_pool(name="ph2", bufs=1) as ph2:
                    sq3 = ph2.tile([C, 3, H + 2, W], F32)
                    nc.scalar.square(out=sq3[:], in_=slab3[:])
                    acc = ph2.tile([C, H, W], F32)
                    nc.vector.tensor_copy(out=acc[:], in_=sq3[:, 0, 0:H, :])
                    for dj in range(3):
                        for di in range(3):
                            if dj == 0 and di == 0:
                                continue
                            nc.vector.tensor_tensor(
                                out=acc[:], in0=acc[:],
                                in1=sq3[:, dj, di:di + H, :], op=OP.add)
                    nrm = ph2.tile([C, LT], F32)
                    for lt in range(LT):
                        tps = pa.tile([C, C], F32, tag="pa", name=f"nt_{lt}")
                        nc.tensor.transpose(
                            tps[:], acc[:, 2 * lt:2 * lt + 2, :], ident[:])
                        tsb = ph2.tile([C, C], F32, tag="ntb")
                        nc.vector.tensor_copy(out=tsb[:], in_=tps[:])
                        nc.vector.tensor_reduce(nrm[:, lt:lt + 1], tsb[:],
                                                axis=AX, op=OP.add)
                    nc.scalar.sqrt(out=nrm[:], in_=nrm[:])
                    nc.vector.reciprocal(out=rnt[:], in_=nrm[:])

                # ---- T tiles: rn * transpose(slab patch rows) -> DRAM
                with tc.tile_pool(name="ph3", bufs=4) as ph3:
                    for lt in range(LT):
                        for k in range(9):
                            di, dj = k // 3, k % 3
                            tps = pa.tile([C, C], F32, tag="pa",
                                          name=f"tt_{lt}_{k}")
                            nc.tensor.transpose(
                                tps[:],
                                slab3[:, dj, 2 * lt + di:2 * lt + di + 2, :],
                                ident[:])
                            tsb = ph3.tile([C, C], F32, tag="tsb")
                            nc.vector.tensor_scalar(
                                out=tsb[:], in0=tps[:],
                                scalar1=rnt[:, lt:lt + 1], scalar2=None,
                                op0=OP.mult)
                            nc.sync.dma_start(
                                out=tv_d[lt, :, k * C:(k + 1) * C], in_=tsb[:])

                # ---- G: H-pass canvases, per-chunk V-pass -> DRAM
                with tc.tile_pool(name="ph4", bufs=1) as ph4:
                    hcv = ph4.tile([C, 3, H + 2, W], F32)
                    nc.vector.memset(hcv[:], 0.0)
                    sdj = ph4.tile([C, H, W], F32, tag="sdj")
                    for dj in range(3):
                        if dj == 1:
                            src = fg32
                        else:
                            src = sdj
                            if dj == 0:
                                nc.vector.memset(sdj[:, :, 0:1], 0.0)
                                nc.vector.tensor_copy(out=sdj[:, :, 1:64],
                                                      in_=fg32[:, :, 0:63])
                            else:
                                nc.vector.memset(sdj[:, :, 63:64], 0.0)
                                nc.vector.tensor_copy(out=sdj[:, :, 0:63],
                                                      in_=fg32[:, :, 1:64])
                        # H-taps (x-position clipped) into canvas rows 1..64
                        nc.vector.tensor_copy(out=hcv[:, dj, 1:H + 1, :],
                                              in_=src[:])
                        nc.vector.tensor_tensor(
                            out=hcv[:, dj, 1:H + 1, 1:64],
                            in0=hcv[:, dj, 1:H + 1, 1:64],
                            in1=src[:, :, 0:63], op=OP.add)
                        nc.vector.tensor_tensor(
                            out=hcv[:, dj, 1:H + 1, 0:63],
                            in0=hcv[:, dj, 1:H + 1, 0:63],
                            in1=src[:, :, 1:64], op=OP.add)
                    with tc.tile_pool(name="gph", bufs=1) as gph:
                        for ch in range(NCHUNK):
                            r0 = 8 * ch
                            gt = gph.tile([C, 9, 8, W], F32, tag="gt")
                            for k in range(9):
                                di, dj = k // 3, k % 3
                                nc.vector.tensor_copy(
                                    out=gt[:, k, :, :],
                                    in_=hcv[:, dj, r0 + di:r0 + di + 8, :])
                                for u in (-1, 1):
                                    dylo = max(0, -u - r0)
                                    dyhi = min(7, 63 - u - r0)
                                    if dylo > dyhi:
                                        continue
                                    nc.vector.tensor_tensor(
                                        out=gt[:, k, dylo:dyhi + 1, :],
                                        in0=gt[:, k, dylo:dyhi + 1, :],
                                        in1=hcv[:, dj,
                                                r0 + dylo + u + di:
                                                r0 + dyhi + u + di + 1, :],
                                        op=OP.add)
                            nc.sync.dma_start(out=g_d[ch, :, :, :, :],
                                              in_=gt[:])

            # ================= main loop over pixel chunks =================
            with (
                tc.tile_pool(name="sspool", bufs=1) as sspool,
                tc.tile_pool(name="gmain", bufs=1) as gmain,
                tc.tile_pool(name="vpool", bufs=3) as vpool,
                tc.tile_pool(name="small", bufs=2) as small,
                tc.tile_pool(name="mo", bufs=4) as mopool,
            ):
                for ch in range(NCHUNK):
                    r0 = 8 * ch
                    gt = gmain.tile([C, 9, 8, W], F32, tag="gt")
                    nc.sync.dma_start(out=gt[:], in_=g_d[ch, :, :, :, :])

                    # matmul1: scores for all 32 l-tiles at this chunk
                    ss = sspool.tile([C, LT, CW], F32, tag="ss")
                    for lt in range(LT):
                        ps = pa.tile([C, CW], F32, tag="pa",
                                     name=f"m1_{ch}_{lt}")
                        for k in range(9):
                            di, dj = k // 3, k % 3
                            lhsT = slab3[:, dj, 2 * lt + di:2 * lt + di + 2, :]
                            nc.tensor.matmul(ps[:], lhsT, gt[:, k, :, :],
                                             start=(k == 0), stop=(k == 8))
                        nc.vector.tensor_scalar(
                            out=ss[:, lt, :], in0=ps[:],
                            scalar1=rnt[:, lt:lt + 1], scalar2=None,
                            op0=OP.mult)

                    # max over l: 32 tiles then across partitions via PE
                    mrun = small.tile([C, CW], F32, tag="mrun")
                    nc.vector.tensor_copy(out=mrun[:], in_=ss[:, 0, :])
                    for lt in range(1, LT):
                        nc.vector.tensor_tensor(out=mrun[:], in0=mrun[:],
                                                in1=ss[:, lt, :], op=OP.max)
                    mb = small.tile([C, CW], F32, tag="mb", name=f"mb_{ch}")
                    for b in range(4):
                        tps = pa.tile([C, C], F32, tag="pa",
                                      name=f"tp_{ch}_{b}")
                        nc.tensor.transpose(tps[:], mrun[:, b * C:(b + 1) * C],
                                            ident[:])
                        tms = small.tile([C, C], F32, tag="tms",
                                         name=f"tms_{ch}_{b}")
                        nc.vector.tensor_copy(out=tms[:], in_=tps[:])
                        mcol = small.tile([C, 1], F32, tag="mcol",
                                          name=f"mc_{ch}_{b}")
                        nc.vector.tensor_reduce(mcol[:], tms[:], axis=AX,
                                                op=OP.max)
                        tp2 = pa.tile([1, C], F32, tag="pa",
                                      name=f"t2_{ch}_{b}")
                        nc.tensor.transpose(tp2[:], mcol[:], ident[:])
                        mrow = small.tile([1, C], F32, tag="mrow",
                                          name=f"mr_{ch}_{b}")
                        nc.vector.tensor_copy(out=mrow[:], in_=tp2[:])
                        bps = pa.tile([C, C], F32, tag="pa",
                                      name=f"bp_{ch}_{b}")
                        nc.tensor.matmul(bps[:], ones1[:], mrow[:],
                                         start=True, stop=True)
                        nc.vector.tensor_copy(out=mb[:, b * C:(b + 1) * C],
                                              in_=bps[:])

                    # E = exp(ss - mb)
                    for lt in range(LT):
                        sl = ss[:, lt, :]
                        nc.vector.tensor_tensor(out=sl, in0=sl, in1=mb[:],
                                                op=OP.subtract)
                        nc.scalar.activation(sl, sl,
                                             mybir.ActivationFunctionType.Exp)

                    # matmul2: out9 planes [c,p] + Z, two PSUM passes
                    mos = []
                    zrow = pz.tile([1, CW], F32, tag="z", name=f"z_{ch}")
                    rzb = None
                    for pass_i, planes in enumerate(((0, 1, 2, 3, 4),
                                                     (5, 6, 7, 8))):
                        c0 = planes[0] * C
                        c1 = (planes[-1] + 1) * C
                        pts = [pb.tile([C, 8, W], F32, tag="o",
                                       name=f"o_{ch}_{j}") for j in planes]
                        for ls in range(LT):
                            vb = vpool.tile([C, 5 * C], F32, tag="vb")
                            nc.sync.dma_start(out=vb[:, :c1 - c0],
                                              in_=tv_d[ls, :, c0:c1])
                            for i, j in enumerate(planes):
                                nc.tensor.matmul(
                                    pts[i][:], vb[:, i * C:(i + 1) * C],
                                    ss[:, ls, :],
                                    start=(ls == 0), stop=(ls == LT - 1))
                            if pass_i == 0:
                                nc.tensor.matmul(zrow[:], onesc[:],
                                                 ss[:, ls, :],
                                                 start=(ls == 0),
                                                 stop=(ls == LT - 1))
                        if pass_i == 0:
                            zs = small.tile([1, CW], F32, tag="zs",
                                            name=f"zs_{ch}")
                            nc.vector.tensor_copy(out=zs[:], in_=zrow[:])
                            rz = small.tile([1, CW], F32, tag="rz",
                                            name=f"rz_{ch}")
                            nc.vector.reciprocal(out=rz[:], in_=zs[:])
                            bps = pa.tile([C, 8, W], F32, tag="pa",
                                          name=f"zb_{ch}")
                            nc.tensor.matmul(bps[:], ones1[:], rz[:],
                                             start=True, stop=True)
                            rzb = small.tile([C, 8, W], F32, tag="rzb",
                                             name=f"rzb_{ch}")
                            nc.vector.tensor_copy(out=rzb[:], in_=bps[:])
                        for i, j in enumerate(planes):
                            mo = mopool.tile([C, 8, W], F32, tag="mo",
                                             name=f"mo_{ch}_{j}")
                            nc.vector.tensor_tensor(out=mo[:], in0=pts[i][:],
                                                    in1=rzb[:], op=OP.mult)
                            mos.append((j, mo))

                    # col2im: rec[y,x] += mo[(di,dj)][y+1-di, x+1-dj]
                    for j, mo in mos:
                        di, dj = j // 3, j % 3
                        ylo = max(0, r0 + di - 1)
                        yhi = min(63, r0 + di + 6)
                        if ylo > yhi:
                            continue
                        sy = ylo - (r0 + di - 1)
                        nr = yhi - ylo + 1
                        if dj == 0:
                            xd, xs, ncol = 0, 1, 63
                        elif dj == 1:
                            xd, xs, ncol = 0, 0, 64
                        else:
                            xd, xs, ncol = 1, 0, 63
                        nc.vector.tensor_tensor(
                            out=rec[:, ylo:ylo + nr, xd:xd + ncol],
                            in0=rec[:, ylo:ylo + nr, xd:xd + ncol],
                            in1=mo[:, sy:sy + nr, xs:xs + ncol],
                            op=OP.add)

                # ---- fp16 cast + output
                out16 = small.tile([C, H, W], F16, tag="o16")
                nc.vector.tensor_copy(out=out16[:], in_=rec[:])
                nc.sync.dma_start(out=out_d[:], in_=out16[:])
    nc.compile()
    return nc


def _make_runner(nc, n_cores):
    """Cached-jit SPMD executor: builds the shard_map wrapper ONCE.

    run_bass_kernel_spmd re-traces + re-jits per call (~650ms overhead under
    axon); this path also creates the donated zero output buffers on device
    (sharded jnp.zeros) instead of uploading them from host each call.
    """
    import jax
    import jax.numpy as jnp
    from jax.sharding import Mesh, PartitionSpec, NamedSharding
    import warnings
    with warnings.catch_warnings():
        warnings.simplefilter("ignore")
        try:
            from jax.experimental.shard_map import shard_map
        except ImportError:
            from jax import shard_map as _sm

            def shard_map(f, **kw):
                kw["check_vma"] = kw.pop("check_rep", False)
                return _sm(f, **kw)
    from concourse.bass2jax import (_bass_exec_p, install_neuronx_cc_hook,
                                    partition_id_tensor)

    install_neuronx_cc_hook()
    partition_name = nc.partition_id_tensor.name if nc.partition_id_tensor else None
    in_names, out_names, out_avals = [], [], []
    for alloc in nc.m.functions[0].allocations:
        if not isinstance(alloc, mybir.MemoryLocationSet):
            continue
        name = alloc.memorylocations[0].name
        if alloc.kind == "ExternalInput":
            if name != partition_name:
                in_names.append(name)
        elif alloc.kind == "ExternalOutput":
            out_names.append(name)
            out_avals.append(jax.core.ShapedArray(
                tuple(alloc.tensor_shape), mybir.dt.np(alloc.dtype)))
    n_params = len(in_names)
    n_outs = len(out_avals)
    all_in_names = list(in_names) + list(out_names)
    if partition_name is not None:
        all_in_names.append(partition_name)

    def _body(*args):
        operands = list(args)
        if partition_name is not None:
            operands.append(partition_id_tensor())
        return tuple(_bass_exec_p.bind(
            *operands, out_avals=tuple(out_avals), in_names=tuple(all_in_names),
            out_names=tuple(out_names), lowering_input_output_aliases=(),
            sim_require_finite=True, sim_require_nnan=True, nc=nc))

    devices = jax.devices()[:n_cores]
    mesh = Mesh(np.asarray(devices), ("core",))
    in_specs = (PartitionSpec("core"),) * (n_params + n_outs)
    out_specs = (PartitionSpec("core"),) * n_outs
    donate = tuple(range(n_params, n_params + n_outs))
    sharded = jax.jit(
        shard_map(_body, mesh=mesh, in_specs=in_specs, out_specs=out_specs,
                  check_rep=False),
        donate_argnums=donate, keep_unused=True)

    zero_shardings = [NamedSharding(mesh, PartitionSpec("core"))] * n_outs

    def _zeros():
        return tuple(jnp.zeros((n_cores * a.shape[0], *a.shape[1:]), a.dtype)
                     for a in out_avals)

    make_zeros = jax.jit(_zeros, out_shardings=tuple(zero_shardings))
    state = {"next_zeros": None}

    def run(in_maps):
        concat_in = [
            np.concatenate([np.asarray(in_maps[c][nm]) for c in range(n_cores)],
                           axis=0) for nm in in_names]
        z = state["next_zeros"] if state["next_zeros"] is not None else make_zeros()
        out_arrs = sharded(*concat_in, *z)
        # async-prepare the next call's donated zero buffers (device-side fill
        # queues behind the exec; done long before the next call arrives)
        state["next_zeros"] = make_zeros()
        out_arrs = [np.asarray(a) for a in out_arrs]
        return [{nm: out_arrs[i].reshape(n_cores, *out_avals[i].shape)[c]
                 for i, nm in enumerate(out_names)} for c in range(n_cores)]

    return run


def make_in_maps(foreground, mask):
    foreground = np.asarray(foreground, np.float32)
    mask = np.asarray(mask, np.float32)
    ident = np.eye(C, dtype=np.float32)
    ones1 = np.ones((1, C), np.float32)
    onesc = np.ones((C, 1), np.float32)
    in_maps = []
    for s in range(foreground.shape[0]):
        in_maps.append({
            "fg16": foreground[s].astype(mybir.dt.np(F8)),
            "m16": mask[s].astype(np.float16),
            "ident": ident,
            "ones1": ones1,
            "onesc": onesc,
        })
    return in_maps


def run_spmd(in_maps):
    """Execute on devices; cached-jit fast path with library fallback."""
    global _compiled, _runner
    if _compiled is None:
        _compiled = _build_program()
    if _runner is None:
        try:
            _runner = _make_runner(_compiled, len(in_maps))
        except Exception:
            _runner = False
    if _runner:
        try:
            return _runner(in_maps)
        except Exception:
            pass
    res = run_bass_kernel_spmd(_compiled, in_maps, list(range(len(in_maps))))
    return res.results


def kernel(foreground, mask, _results_hook=None):
    foreground = np.asarray(foreground, np.float32)
    mask = np.asarray(mask, np.float32)
    B = foreground.shape[0]

    in_maps = make_in_maps(foreground, mask)
    results = run_spmd(in_maps)

    out = np.empty_like(foreground)
    for s in range(B):
        rec = np.asarray(results[s]["out"]).astype(np.float32)
        m = mask[s]
        out[s] = rec * m / 9.0 + foreground[s] * (1.0 - m)
    return out
